# revision 14
# baseline (speedup 1.0000x reference)
"""ObjectAttentionBlock2D TRN2 kernel.

Reference computation (per batch b):
    xf    = x[b].reshape(C, N)                  # C=512, N=128*128=16384
    pf    = proxy[b,:,:,0]                      # [C, K], K=64
    query = Wq @ xf + bq                        # [Ck=256, N]
    keym  = Wk @ pf + bk                        # [Ck, K]
    value = (Wv @ pf + bv).T                    # [K, Cv=256]
    sim   = softmax_k(query.T @ keym / 16)      # [N, K]
    ctx   = sim @ value                         # [N, Cv]
    out   = Wo @ ctx.T + bo                     # [C, N]

Sharding: data-parallel over batch. B=8 batches -> 8 NeuronCores, one image
per core, no collectives. Weights are replicated (host pre-transposes them so
the contraction dim is the SBUF partition dim).

Key algebraic optimization: the attention-logit and output maps are both
rank-K (K=64), and query/ctx each feed exactly one matmul, so both
projections fold into small precomputed matrices (on-device, per core):
  M     = Wq^T @ keym            [C, K]   -> simT = M^T x (4 MMs, was 10)
  sbias = (bq/16)^T @ keym       [K, 1]   -> rides in exp's bias slot
  WVT   = (Wo @ value^T)^T       [K, C]   -> out = WVT^T expPn (4 MMs, was 10)

The kernel is DMA-bound (in the TimelineSim cost model every DMA serializes
on one 360 GB/s DMA-engine pool), so both HBM streams are compressed:
  - x  is fp16 (halves the input stream; 10-bit mantissa suffices; fp8
    x was measured at 1.7e-2 end-to-end error vs the 2e-2 gate - too
    close, because logit quantization noise is amplified by softmax).
  - out is uint8 with an exact per-channel scale: out[c,:] is a convex
    combination (softmax weights) of WVT[:,c] entries plus bo[c], so
    bound_c = max_k |WVT[k,c] + bo[c]| bounds |out[c,:]| EXACTLY. The
    device computes bound_c (8 extra tiny matmuls give WVT^T in the
    partition layout of the output, then a free-axis abs-max), quantizes
    u8 = s_c*out + s_c*bo + 128 in the output-stage scale/bias slots
    (s_c = 126.5/bound_c), and ships sinv_c = bound_c/126.5 back; the
    host dequantizes (u8 - 128) * sinv. HW converts f32->u8 with
    round-to-nearest (verified empirically: offset 128.0 beats 127.5/
    128.5 2x). Output quantization adds only bounded absolute error (no
    softmax amplification): measured end-to-end rel err ~6.5e-3.

Engine-cost model (TimelineSim): every ACT/DVE instruction costs
~125-185ns fixed + ~1ns/free-elem, and a matmul costs out_free_size x
0.44ns regardless of contraction rows. At F=256 the per-instruction
fixed costs made DVE (104us) and ACT (78us) the critical path, so the
pipeline runs F=512 tiles: per 512-px tile 10 MMs, 1 exp, 1 recip, 1
en-mult, and the 4 quantize chunks are spread ACT/ACT/DVE/Pool to
balance engines. Predicted busy: DMA 74us > PE ~69 > DVE ~63 > Pool
~60 > ACT ~59 -> DMA-bound again.

Per-core pipeline over 32 macro-tiles of MF=512 pixels:
  x DMA [128, 4, 512] fp16 on gpsimd/SWDGE (1024B descriptors; >=512B
    dodges the sub-512B read-modify-write 2x penalty, and one dispatch
    per 512 px halves SWDGE descriptor-generation time on Pool)
  simT [K=64, 512] (4 fp16 MMs, contract C=512, lhsT=M)
  ACT exp(sim/16 + sbias) -> f32r SBUF
  denom = ones64^T @ expP (1 MM) -> DVE reciprocal -> K=1 broadcast MM
  expPn = expP * recip (DVE)
  out [C, 512] (4 f32r MMs, contract K=64, lhsT=WVT) -> quantize 4
    chunks (ACT/ACT/DVE/Pool) -> u8 SBUF -> one DMA on the SP HWDGE
    queue (512B descriptors)
PSUM: sim 2 bufs x 1 bank + den 1 + rb 1 + outps 1 buf x 4 banks = 8.
keym/V2/M/sbias/WVT/bounds are precomputed once per core with biases
folded in via K=1 matmul accumulation (bias outer-product with ones).

Precision: x/Wq/Wk/Wv/Wo/pf are cast to fp16 on the host; everything
downstream runs float32r (1 cycle/row at free>=256) with fp32 PSUM
accumulation. Output u8 + per-channel scale as above.
"""

import numpy as np

import concourse.bacc as bacc
import concourse.mybir as mybir
import concourse.tile as tile
from concourse import bass_utils

F32 = mybir.dt.float32
F32R = mybir.dt.float32r
F16 = mybir.dt.float16
U8 = mybir.dt.uint8

B, C, H, W = 8, 512, 128, 128
N = H * W                    # 16384 pixels per image
CK, CV, K = 256, 256, 64
P = 128                      # SBUF partitions
MF = 512                     # pixel tile width (compute + DMA)
NMT = N // MF                # 32 tiles
CI_CH = C // P               # 4 contraction chunks over C
Q_CH = CK // P               # 2 chunks over Ck
V_CH = CV // P               # 2 chunks over Cv
O_CH = C // P                # 4 chunks over output C
SCALE = CK ** -0.5           # 1/16
QCAP = 126.5                 # |s*out| <= 126.5 so u8 = s*out+128 in [1.5, 254.5]

_CACHED = None


def _build():
    nc = bacc.Bacc("TRN2", target_bir_lowering=False, debug=False)

    X = nc.dram_tensor("x", [C, N], F16, kind="ExternalInput").ap()
    # pack16[c, :] = [pf(64) | wkT(256) | wvT(256)] in fp16
    PACK16 = nc.dram_tensor("pack16", [C, 576], F16, kind="ExternalInput").ap()
    WQ = nc.dram_tensor("wq", [CK, C], F16, kind="ExternalInput").ap()
    # crow = [bk(256) | bv(256) | ones(256) | bo(512)] as one row
    CROW = nc.dram_tensor("crow", [1, 1280], F32, kind="ExternalInput").ap()
    ONESC = nc.dram_tensor("ones_col", [K, 1], F32, kind="ExternalInput").ap()
    # bqbo[p, :] = [bq 2 chunks | bo 4 chunks] per-partition layout
    BQBO = nc.dram_tensor("bqbo", [P, 6], F32, kind="ExternalInput").ap()
    BQS16 = nc.dram_tensor("bqs16", [P, 2], F16, kind="ExternalInput").ap()
    WOT = nc.dram_tensor("woT", [CV, C], F16, kind="ExternalInput").ap()
    OUT = nc.dram_tensor("out", [C, N], U8, kind="ExternalOutput").ap()
    SINV = nc.dram_tensor("sinv", [1, C], F32, kind="ExternalOutput").ap()

    x_r = X.rearrange("(co p) n -> p co n", p=P)       # [128, 4, N]
    out_r = OUT.rearrange("(oo p) n -> p oo n", p=P)   # [128, 4, N]

    with tile.TileContext(nc) as tc:
        with tc.tile_pool(name="const", bufs=1) as cp:
            pack = cp.tile([P, CI_CH, 576], F16)
            nc.sync.dma_start(pack, PACK16.rearrange("(co p) q -> p co q", p=P))
            pf = pack[:, :, 0:K]
            wk = pack[:, :, K:K + CK]
            wv = pack[:, :, K + CK:K + CK + CV]
            wq = cp.tile([P, Q_CH, C], F16)
            nc.sync.dma_start(wq, WQ.rearrange("(qo p) c -> p qo c", p=P))
            crow = cp.tile([1, 1280], F32R)
            nc.sync.dma_start(crow, CROW.bitcast(F32R))
            bk_row = crow[:, 0:CK]
            bv_row = crow[:, CK:CK + CV]
            ones_row = crow[:, 512:768]
            bo_row = crow[:, 768:1280]
            ones_col = cp.tile([K, 1], F32R)
            nc.sync.dma_start(ones_col, ONESC.bitcast(F32R))
            bqbo = cp.tile([P, 6], F32)
            nc.scalar.dma_start(bqbo, BQBO)
            bqs16 = cp.tile([P, 2], F16)
            nc.scalar.dma_start(bqs16, BQS16)
            bqs = bqs16
            bo = bqbo[:, 2:6]
            wo = cp.tile([P, V_CH, C], F16)
            nc.scalar.dma_start(wo, WOT.rearrange("(vo p) o -> p vo o", p=P))

            keym = cp.tile([P, Q_CH, K], F16)    # [q-part, q-chunk, k]
            msim = cp.tile([P, CI_CH, K], F16)   # M[c,k] = sum_q Wq[q,c]*keym[q,k]
            sbias = cp.tile([K, 1], F32)         # sum_q (bq[q]/16)*keym[q,k]
            # u8 quantization: out[c,:] is a convex combination (softmax
            # weights sum to 1) of WVT[:,c]+bo[c], so with
            #   wvtq[k,c] = s_c*(WVT[k,c]+bo[c]) + 128,  s_c = 126.5/bound_c
            # the out matmul directly yields u8-domain values in [1.5,254.5]
            # (the +128 rides on sum_k en = 1) and the output stage is a pure
            # f32->u8 copy with no per-channel scalars.
            wvtb = cp.tile([K, C], F32R)         # WVT[k,c] + bo[c]
            wvtq = cp.tile([K, C], F32R)         # s_c*(WVT+bo) + 128
            bound_row = cp.tile([1, C], F32R)    # max_k |WVT[k,c]+bo[c]|
            r_row = cp.tile([1, C], F32R)
            s_row = cp.tile([1, C], F32R)        # 126.5 / bound
            sinv_row = cp.tile([1, C], F32R)     # bound / 126.5

            # ---- one-time: keym = Wk @ pf + bk, value[k,v] = (Wv @ pf + bv)[v,k]
            with tc.tile_pool(name="setup_ps", bufs=1, space="PSUM") as sps:
                kps = sps.tile([P, Q_CH, K], F32)
                for qi in range(Q_CH):
                    for ci in range(CI_CH):
                        nc.tensor.matmul(
                            kps[:, qi, :],
                            wk[:, ci, qi * P:(qi + 1) * P],
                            pf[:, ci, :],
                            start=(ci == 0), stop=False,
                        )
                    # += bk[q] * ones[k]
                    nc.tensor.matmul(
                        kps[:, qi, :],
                        bk_row[:, qi * P:(qi + 1) * P],
                        ones_row[:, :K],
                        start=False, stop=True,
                    )
                nc.vector.tensor_copy(keym, kps)

                v2ps = sps.tile([P, V_CH, K], F32)
                for vi in range(V_CH):
                    for ci in range(CI_CH):
                        nc.tensor.matmul(
                            v2ps[:, vi, :],
                            wv[:, ci, vi * P:(vi + 1) * P],
                            pf[:, ci, :],
                            start=(ci == 0), stop=False,
                        )
                    nc.tensor.matmul(
                        v2ps[:, vi, :],
                        bv_row[:, vi * P:(vi + 1) * P],
                        ones_row[:, :K],
                        start=False, stop=True,
                    )
                v2sb = cp.tile([P, V_CH, K], F16)
                nc.vector.tensor_copy(v2sb, v2ps)
                wvtps = sps.tile([K, C], F32)
                for vi in range(V_CH):
                    nc.tensor.matmul(
                        wvtps, v2sb[:, vi, :], wo[:, vi, :],
                        start=(vi == 0), stop=False,
                    )
                # += bo broadcast along k (rank-1 accumulate) -> WVT + bo
                nc.tensor.matmul(
                    wvtps, ones_row[:, :K], bo_row, start=False, stop=True,
                )
                with nc.allow_low_precision(reason="f32r is 4-byte fp32"):
                    nc.vector.tensor_copy(wvtb, wvtps)
                    # exact |out| bound per channel: abs on DVE (gpsimd's
                    # apply_absolute_value is ignored on HW), then
                    # cross-partition max on gpsimd
                    wvta = cp.tile([K, C], F32R)
                    nc.vector.tensor_scalar(
                        wvta, wvtb, -1.0, None, op0=mybir.AluOpType.mult,
                    )
                    nc.vector.tensor_tensor(wvta, wvtb, wvta, mybir.AluOpType.max)
                    nc.gpsimd.tensor_reduce(
                        bound_row, wvta, axis=mybir.AxisListType.C,
                        op=mybir.AluOpType.max,
                    )
                    nc.vector.tensor_scalar(
                        bound_row, bound_row, 1e-3, None, op0=mybir.AluOpType.max,
                    )
                    nc.vector.reciprocal(r_row, bound_row)
                    nc.vector.tensor_scalar(
                        s_row, r_row, QCAP, None, op0=mybir.AluOpType.mult,
                    )
                    nc.vector.tensor_scalar(
                        sinv_row, bound_row, 1.0 / QCAP, None,
                        op0=mybir.AluOpType.mult,
                    )
                    nc.scalar.dma_start(SINV, sinv_row.bitcast(F32))
                    # wvtq = s*(WVT+bo) + 128, via rank-1 broadcast of s
                    s_bc = sps.tile([K, C], F32)
                    nc.tensor.matmul(s_bc, ones_row[:, :K], s_row, start=True, stop=True)
                    nc.vector.tensor_tensor(wvtq, wvtb, s_bc, mybir.AluOpType.mult)
                    nc.vector.tensor_scalar(
                        wvtq, wvtq, 128.0, None, op0=mybir.AluOpType.add,
                    )

                # M: fold the Q projection into the sim matmul (Q only feeds sim)
                mps = sps.tile([P, CI_CH, K], F32)
                for ci in range(CI_CH):
                    for qi in range(Q_CH):
                        nc.tensor.matmul(
                            mps[:, ci, :],
                            wq[:, qi, ci * P:(ci + 1) * P],
                            keym[:, qi, :],
                            start=(qi == 0), stop=(qi == Q_CH - 1),
                        )
                nc.vector.tensor_copy(msim, mps)
                # sbias[k,1]: lhsT=keym chunks, rhs=bq/16 column
                sbps = sps.tile([K, 1], F32)
                for qi in range(Q_CH):
                    nc.tensor.matmul(
                        sbps, keym[:, qi, :], bqs[:, qi:qi + 1],
                        start=(qi == 0), stop=(qi == Q_CH - 1),
                    )
                nc.vector.tensor_copy(sbias, sbps)

            # ---- steady-state pipeline over 512-px tiles
            with (
                tc.tile_pool(name="xin", bufs=5) as xp,
                tc.tile_pool(name="esb", bufs=4) as ep,
                tc.tile_pool(name="rsb", bufs=4) as rp,
                tc.tile_pool(name="ensb", bufs=4) as enp,
                tc.tile_pool(name="outsb", bufs=4) as outp,
                tc.tile_pool(name="sdps", bufs=2, space="PSUM") as sdps,
                tc.tile_pool(name="denps", bufs=1, space="PSUM") as denps,
                tc.tile_pool(name="rbps", bufs=1, space="PSUM") as rbps,
                tc.tile_pool(name="outps", bufs=1, space="PSUM") as outps,
            ):
                for mt in range(NMT):
                    m0 = mt * MF
                    x_t = xp.tile([P, CI_CH, MF], F16, tag="x")
                    nc.gpsimd.dma_start(x_t, x_r[:, :, m0:m0 + MF])

                    # simT[k, n] = M^T-contract-c @ x (Q proj folded into M)
                    sim = sdps.tile([K, MF], F32, tag="sd")
                    den = denps.tile([1, MF], F32, tag="den")
                    for ci in range(CI_CH):
                        nc.tensor.matmul(
                            sim, msim[:, ci, :], x_t[:, ci, :],
                            start=(ci == 0), stop=(ci == CI_CH - 1),
                        )
                    e = ep.tile([K, MF], F32R, tag="e")
                    nc.scalar.activation(
                        e, sim, mybir.ActivationFunctionType.Exp,
                        scale=SCALE, bias=sbias,
                    )
                    nc.tensor.matmul(den, ones_col, e, start=True, stop=True)
                    r_sb = rp.tile([1, MF], F32R, tag="r")
                    with nc.allow_low_precision(reason="f32r is 4-byte fp32"):
                        nc.vector.reciprocal(r_sb, den)
                    rb_ps = rbps.tile([K, MF], F32, tag="rb")
                    nc.tensor.matmul(rb_ps, ones_row[:, :K], r_sb, start=True, stop=True)
                    en = enp.tile([K, MF], F32R, tag="en")
                    nc.vector.tensor_tensor(en, rb_ps, e, mybir.AluOpType.mult)

                    # out matmul directly in the u8 domain (scale+bias folded
                    # into wvtq); output stage is a pure f32->u8 copy, split
                    # 3 chunks on ACT / 1 on DVE to balance engine load
                    out_ps = outps.tile([P, O_CH, MF], F32, tag="outps")
                    out_u8 = outp.tile([P, O_CH, MF], U8, tag="out")
                    for oi in range(O_CH):
                        nc.tensor.matmul(
                            out_ps[:, oi, :],
                            wvtq[:, oi * P:(oi + 1) * P],
                            en,
                            start=True, stop=True,
                        )
                    nc.scalar.activation(
                        out_u8[:, 0:3, :], out_ps[:, 0:3, :],
                        mybir.ActivationFunctionType.Identity,
                    )
                    nc.vector.tensor_copy(out_u8[:, 3, :], out_ps[:, 3, :])
                    nc.sync.dma_start(out_r[:, :, m0:m0 + MF], out_u8)

    nc.compile()
    return nc


def _get_nc():
    global _CACHED
    if _CACHED is None:
        _CACHED = _build()
    return _CACHED


def kernel(x, proxy, Wq, bq, Wk, bk, Wv, bv, Wo, bo, **run_kwargs):
    nc = _get_nc()

    crow = np.concatenate(
        [np.asarray(bk, np.float32).reshape(1, CK),
         np.asarray(bv, np.float32).reshape(1, CV),
         np.ones((1, 256), np.float32),
         np.asarray(bo, np.float32).reshape(1, C)], axis=1)
    bqbo = np.concatenate(
        [np.asarray(bq, np.float32).reshape(2, P).T,
         np.asarray(bo, np.float32).reshape(4, P).T], axis=1)
    w16 = np.concatenate(
        [np.asarray(Wk).T, np.asarray(Wv).T], axis=1
    ).astype(np.float16)
    shared = {
        "woT": np.ascontiguousarray(Wo.T).astype(np.float16),
        "wq": np.ascontiguousarray(Wq).astype(np.float16),
        "bqs16": np.ascontiguousarray(
            (np.asarray(bq, np.float32) * SCALE).reshape(2, P).T
        ).astype(np.float16),
        "crow": np.ascontiguousarray(crow),
        "bqbo": np.ascontiguousarray(bqbo),
        "ones_col": np.ones((K, 1), np.float32),
    }
    in_maps = []
    for b in range(B):
        m = dict(shared)
        m["x"] = np.ascontiguousarray(x[b]).reshape(C, N).astype(np.float16)
        pf16 = np.asarray(proxy[b, :, :, 0]).astype(np.float16)
        m["pack16"] = np.ascontiguousarray(np.concatenate([pf16, w16], axis=1))
        in_maps.append(m)

    res = bass_utils.run_bass_kernel_spmd(
        nc, in_maps, core_ids=list(range(B)), **run_kwargs
    )
    kernel.last_results = res
    out = np.empty((B, C, N), np.float32)
    for b in range(B):
        u8 = res.results[b]["out"].astype(np.float32)
        sinv = np.asarray(res.results[b]["sinv"], np.float32).reshape(C)
        out[b] = (u8 - 128.0) * sinv[:, None]
    return out.reshape(B, C, H, W)


# revision 16
# speedup vs baseline: 1.0762x; 1.0762x over previous
"""ObjectAttentionBlock2D TRN2 kernel.

Reference computation (per batch b):
    xf    = x[b].reshape(C, N)                  # C=512, N=128*128=16384
    pf    = proxy[b,:,:,0]                      # [C, K], K=64
    query = Wq @ xf + bq                        # [Ck=256, N]
    keym  = Wk @ pf + bk                        # [Ck, K]
    value = (Wv @ pf + bv).T                    # [K, Cv=256]
    sim   = softmax_k(query.T @ keym / 16)      # [N, K]
    ctx   = sim @ value                         # [N, Cv]
    out   = Wo @ ctx.T + bo                     # [C, N]

Sharding: data-parallel over batch. B=8 batches -> 8 NeuronCores, one image
per core, no collectives. Weights are replicated (host pre-transposes them so
the contraction dim is the SBUF partition dim).

Key algebraic optimization: the attention-logit and output maps are both
rank-K (K=64), and query/ctx each feed exactly one matmul, so both
projections fold into small precomputed matrices (on-device, per core):
  M     = Wq^T @ keym            [C, K]   -> simT = M^T x (4 MMs, was 10)
  sbias = (bq/16)^T @ keym       [K, 1]   -> rides in exp's bias slot
  WVT   = (Wo @ value^T)^T       [K, C]   -> out = WVT^T expPn (4 MMs, was 10)

The kernel is DMA-bound (in the TimelineSim cost model every DMA serializes
on one 360 GB/s DMA-engine pool), so both HBM streams are compressed:
  - x  is fp16 (halves the input stream; 10-bit mantissa suffices; fp8
    x was measured at 1.7e-2 end-to-end error vs the 2e-2 gate - too
    close, because logit quantization noise is amplified by softmax).
  - out is uint8 with an exact per-channel scale: out[c,:] is a convex
    combination (softmax weights) of WVT[:,c] entries plus bo[c], so
    bound_c = max_k |WVT[k,c] + bo[c]| bounds |out[c,:]| EXACTLY. The
    device computes bound_c (8 extra tiny matmuls give WVT^T in the
    partition layout of the output, then a free-axis abs-max), quantizes
    u8 = s_c*out + s_c*bo + 128 in the output-stage scale/bias slots
    (s_c = 126.5/bound_c), and ships sinv_c = bound_c/126.5 back; the
    host dequantizes (u8 - 128) * sinv. HW converts f32->u8 with
    round-to-nearest (verified empirically: offset 128.0 beats 127.5/
    128.5 2x). Output quantization adds only bounded absolute error (no
    softmax amplification): measured end-to-end rel err ~6.5e-3.

Engine-cost model (TimelineSim): every ACT/DVE instruction costs
~125-185ns fixed + ~1ns/free-elem, and a matmul costs out_free_size x
0.44ns regardless of contraction rows. At F=256 the per-instruction
fixed costs made DVE (104us) and ACT (78us) the critical path, so the
pipeline runs F=512 tiles: per 512-px tile 10 MMs, 1 exp, 1 recip, 1
en-mult, and the 4 quantize chunks are spread ACT/ACT/DVE/Pool to
balance engines. Predicted busy: DMA 74us > PE ~69 > DVE ~63 > Pool
~60 > ACT ~59 -> DMA-bound again.

Per-core pipeline over 32 macro-tiles of MF=512 pixels:
  x DMA [128, 4, 512] fp16 on gpsimd/SWDGE (1024B descriptors; >=512B
    dodges the sub-512B read-modify-write 2x penalty, and one dispatch
    per 512 px halves SWDGE descriptor-generation time on Pool)
  simT [K=64, 512] (4 fp16 MMs, contract C=512, lhsT=M)
  ACT exp(sim/16 + sbias) -> f32r SBUF
  denom = ones64^T @ expP (1 MM) -> DVE reciprocal -> K=1 broadcast MM
  expPn = expP * recip (DVE)
  out [C, 512] (4 f32r MMs, contract K=64, lhsT=WVT) -> quantize 4
    chunks (ACT/ACT/DVE/Pool) -> u8 SBUF -> one DMA on the SP HWDGE
    queue (512B descriptors)
PSUM: sim 2 bufs x 1 bank + den 1 + rb 1 + outps 1 buf x 4 banks = 8.
keym/V2/M/sbias/WVT/bounds are precomputed once per core with biases
folded in via K=1 matmul accumulation (bias outer-product with ones).

Precision: x/Wq/Wk/Wv/Wo/pf are cast to fp16 on the host; everything
downstream runs float32r (1 cycle/row at free>=256) with fp32 PSUM
accumulation. Output u8 + per-channel scale as above.
"""

import numpy as np

import concourse.bacc as bacc
import concourse.mybir as mybir
import concourse.tile as tile
from concourse import bass_utils

F32 = mybir.dt.float32
F32R = mybir.dt.float32r
F16 = mybir.dt.float16
U8 = mybir.dt.uint8

B, C, H, W = 8, 512, 128, 128
N = H * W                    # 16384 pixels per image
CK, CV, K = 256, 256, 64
P = 128                      # SBUF partitions
MF = 512                     # pixel tile width (compute + DMA)
NMT = N // MF                # 32 tiles
CI_CH = C // P               # 4 contraction chunks over C
Q_CH = CK // P               # 2 chunks over Ck
V_CH = CV // P               # 2 chunks over Cv
O_CH = C // P                # 4 chunks over output C
SCALE = CK ** -0.5           # 1/16
QCAP = 126.5                 # |s*out| <= 126.5 so u8 = s*out+128 in [1.5, 254.5]

_CACHED = None


def _build():
    nc = bacc.Bacc("TRN2", target_bir_lowering=False, debug=False)

    X = nc.dram_tensor("x", [C, N], F16, kind="ExternalInput").ap()
    # pack16[c, :] = [pf(64) | wkT(256) | wvT(256)] in fp16
    PACK16 = nc.dram_tensor("pack16", [C, 576], F16, kind="ExternalInput").ap()
    WQ = nc.dram_tensor("wq", [CK, C], F16, kind="ExternalInput").ap()
    # crow = [bk(256) | bv(256) | ones(256) | bo(512)] as one row
    CROW = nc.dram_tensor("crow", [1, 1280], F32, kind="ExternalInput").ap()
    ONESC = nc.dram_tensor("ones_col", [K, 1], F32, kind="ExternalInput").ap()
    # bqbo[p, :] = [bq 2 chunks | bo 4 chunks] per-partition layout
    BQBO = nc.dram_tensor("bqbo", [P, 6], F32, kind="ExternalInput").ap()
    BQS16 = nc.dram_tensor("bqs16", [P, 2], F16, kind="ExternalInput").ap()
    WOT = nc.dram_tensor("woT", [CV, C], F16, kind="ExternalInput").ap()
    OUT = nc.dram_tensor("out", [C, N], U8, kind="ExternalOutput").ap()
    SINV = nc.dram_tensor("sinv", [1, C], F32, kind="ExternalOutput").ap()

    x_r = X.rearrange("(co p) n -> p co n", p=P)       # [128, 4, N]
    out_r = OUT.rearrange("(oo p) n -> p oo n", p=P)   # [128, 4, N]

    with tile.TileContext(nc) as tc:
        with tc.tile_pool(name="const", bufs=1) as cp:
            pack = cp.tile([P, CI_CH, 576], F16)
            nc.sync.dma_start(pack, PACK16.rearrange("(co p) q -> p co q", p=P))
            pf = pack[:, :, 0:K]
            wk = pack[:, :, K:K + CK]
            wv = pack[:, :, K + CK:K + CK + CV]
            wq = cp.tile([P, Q_CH, C], F16)
            nc.sync.dma_start(wq, WQ.rearrange("(qo p) c -> p qo c", p=P))
            crow = cp.tile([1, 1280], F32R)
            nc.sync.dma_start(crow, CROW.bitcast(F32R))
            bk_row = crow[:, 0:CK]
            bv_row = crow[:, CK:CK + CV]
            ones_row = crow[:, 512:768]
            bo_row = crow[:, 768:1280]
            ones_col = cp.tile([K, 1], F32R)
            nc.sync.dma_start(ones_col, ONESC.bitcast(F32R))
            bqbo = cp.tile([P, 6], F32)
            nc.scalar.dma_start(bqbo, BQBO)
            bqs16 = cp.tile([P, 2], F16)
            nc.scalar.dma_start(bqs16, BQS16)
            bqs = bqs16
            bo = bqbo[:, 2:6]
            wo = cp.tile([P, V_CH, C], F16)
            nc.scalar.dma_start(wo, WOT.rearrange("(vo p) o -> p vo o", p=P))

            keym = cp.tile([P, Q_CH, K], F16)    # [q-part, q-chunk, k]
            msim = cp.tile([P, CI_CH, K], F16)   # M[c,k] = sum_q Wq[q,c]*keym[q,k]
            sbias = cp.tile([K, 1], F32)         # sum_q (bq[q]/16)*keym[q,k]
            # u8 quantization: out[c,:] is a convex combination (softmax
            # weights sum to 1) of WVT[:,c]+bo[c], so with
            #   wvtq[k,c] = s_c*(WVT[k,c]+bo[c]) + 128,  s_c = 126.5/bound_c
            # the out matmul directly yields u8-domain values in [1.5,254.5]
            # (the +128 rides on sum_k en = 1) and the output stage is a pure
            # f32->u8 copy with no per-channel scalars.
            wvtb = cp.tile([K, C], F32R)         # WVT[k,c] + bo[c]
            wvtq = cp.tile([K, C], F32R)         # s_c*(WVT+bo) + 128
            bound_row = cp.tile([1, C], F32R)    # max_k |WVT[k,c]+bo[c]|
            r_row = cp.tile([1, C], F32R)
            s_row = cp.tile([1, C], F32R)        # 126.5 / bound
            sinv_row = cp.tile([1, C], F32R)     # bound / 126.5

            # ---- one-time: keym = Wk @ pf + bk, value[k,v] = (Wv @ pf + bv)[v,k]
            with tc.tile_pool(name="setup_ps", bufs=1, space="PSUM") as sps:
                kps = sps.tile([P, Q_CH, K], F32)
                for qi in range(Q_CH):
                    for ci in range(CI_CH):
                        nc.tensor.matmul(
                            kps[:, qi, :],
                            wk[:, ci, qi * P:(qi + 1) * P],
                            pf[:, ci, :],
                            start=(ci == 0), stop=False,
                        )
                    # += bk[q] * ones[k]
                    nc.tensor.matmul(
                        kps[:, qi, :],
                        bk_row[:, qi * P:(qi + 1) * P],
                        ones_row[:, :K],
                        start=False, stop=True,
                    )
                nc.vector.tensor_copy(keym, kps)

                v2ps = sps.tile([P, V_CH, K], F32)
                for vi in range(V_CH):
                    for ci in range(CI_CH):
                        nc.tensor.matmul(
                            v2ps[:, vi, :],
                            wv[:, ci, vi * P:(vi + 1) * P],
                            pf[:, ci, :],
                            start=(ci == 0), stop=False,
                        )
                    nc.tensor.matmul(
                        v2ps[:, vi, :],
                        bv_row[:, vi * P:(vi + 1) * P],
                        ones_row[:, :K],
                        start=False, stop=True,
                    )
                v2sb = cp.tile([P, V_CH, K], F16)
                nc.vector.tensor_copy(v2sb, v2ps)
                wvtps = sps.tile([K, C], F32)
                for vi in range(V_CH):
                    nc.tensor.matmul(
                        wvtps, v2sb[:, vi, :], wo[:, vi, :],
                        start=(vi == 0), stop=False,
                    )
                # += bo broadcast along k (rank-1 accumulate) -> WVT + bo
                nc.tensor.matmul(
                    wvtps, ones_row[:, :K], bo_row, start=False, stop=True,
                )
                with nc.allow_low_precision(reason="f32r is 4-byte fp32"):
                    nc.vector.tensor_copy(wvtb, wvtps)
                    # exact |out| bound per channel: abs on DVE (gpsimd's
                    # apply_absolute_value is ignored on HW), then
                    # cross-partition max on gpsimd
                    wvta = cp.tile([K, C], F32R)
                    nc.vector.tensor_scalar(
                        wvta, wvtb, -1.0, None, op0=mybir.AluOpType.mult,
                    )
                    nc.vector.tensor_tensor(wvta, wvtb, wvta, mybir.AluOpType.max)
                    nc.gpsimd.tensor_reduce(
                        bound_row, wvta, axis=mybir.AxisListType.C,
                        op=mybir.AluOpType.max,
                    )
                    nc.vector.tensor_scalar(
                        bound_row, bound_row, 1e-3, None, op0=mybir.AluOpType.max,
                    )
                    nc.vector.reciprocal(r_row, bound_row)
                    nc.vector.tensor_scalar(
                        s_row, r_row, QCAP, None, op0=mybir.AluOpType.mult,
                    )
                    nc.vector.tensor_scalar(
                        sinv_row, bound_row, 1.0 / QCAP, None,
                        op0=mybir.AluOpType.mult,
                    )
                    nc.scalar.dma_start(SINV, sinv_row.bitcast(F32))
                    # wvtq = s*(WVT+bo) + 128, via rank-1 broadcast of s
                    s_bc = sps.tile([K, C], F32)
                    nc.tensor.matmul(s_bc, ones_row[:, :K], s_row, start=True, stop=True)
                    nc.vector.tensor_tensor(wvtq, wvtb, s_bc, mybir.AluOpType.mult)
                    nc.vector.tensor_scalar(
                        wvtq, wvtq, 128.0, None, op0=mybir.AluOpType.add,
                    )

                # M: fold the Q projection into the sim matmul (Q only feeds sim)
                mps = sps.tile([P, CI_CH, K], F32)
                for ci in range(CI_CH):
                    for qi in range(Q_CH):
                        nc.tensor.matmul(
                            mps[:, ci, :],
                            wq[:, qi, ci * P:(ci + 1) * P],
                            keym[:, qi, :],
                            start=(qi == 0), stop=(qi == Q_CH - 1),
                        )
                nc.vector.tensor_copy(msim, mps)
                # sbias[k,1]: lhsT=keym chunks, rhs=bq/16 column
                sbps = sps.tile([K, 1], F32)
                for qi in range(Q_CH):
                    nc.tensor.matmul(
                        sbps, keym[:, qi, :], bqs[:, qi:qi + 1],
                        start=(qi == 0), stop=(qi == Q_CH - 1),
                    )
                nc.vector.tensor_copy(sbias, sbps)

            # ---- steady-state: 4-stage software pipeline over 512-px tiles
            # Iteration i runs  S0: sim+exp(i) | S1: den+recip(i-1) |
            # S2: rb+en(i-2) | S3: out MMs + u8 copies + DMA(i-3).
            # The softmax chain has 3 cross-engine round trips; skewing the
            # stages gives every dependency a full iteration of slack so each
            # in-order engine queue never waits (the naive fused loop ran all
            # engines at ~50% on exactly those round trips).
            # Copy split tuned to balance ACT (exp + 1038 + 532 = 2182ns/it)
            # vs DVE (recip + en + 658 + 225 = 2199ns/it), both under the
            # 2287ns/it DMA floor.
            PRE = 2   # x-DMA prefetch distance (iterations)
            with (
                tc.tile_pool(name="xin", bufs=5) as xp,
                tc.tile_pool(name="esb", bufs=4) as ep,
                tc.tile_pool(name="rsb", bufs=3) as rp,
                tc.tile_pool(name="ensb", bufs=3) as enp,
                tc.tile_pool(name="outsb", bufs=3) as outp,
                tc.tile_pool(name="sdps", bufs=1, space="PSUM") as sdps,
                tc.tile_pool(name="denps", bufs=2, space="PSUM") as denps,
                tc.tile_pool(name="rbps", bufs=1, space="PSUM") as rbps,
                tc.tile_pool(name="outaps", bufs=1, space="PSUM") as outaps,
                tc.tile_pool(name="outbps", bufs=1, space="PSUM") as outbps,
            ):
                xt = {}
                et = {}
                rt = {}
                dent = {}
                rbt = {}
                ent = {}

                def fetch_x(t):
                    if t < NMT:
                        xt[t] = xp.tile([P, CI_CH, MF], F16, tag="x", name=f"x{t}")
                        nc.gpsimd.dma_start(xt[t], x_r[:, :, t * MF:(t + 1) * MF])

                for t in range(PRE):
                    fetch_x(t)

                for i in range(NMT + 3):
                    fetch_x(i + PRE)
                    t0, t1, t2, t3 = i, i - 1, i - 2, i - 3
                    if t0 < NMT:
                        # S0: simT[k,n] = M^T-contract-c @ x, then exp
                        sim = sdps.tile([K, MF], F32, tag="sd")
                        for ci in range(CI_CH):
                            nc.tensor.matmul(
                                sim, msim[:, ci, :], xt[t0][:, ci, :],
                                start=(ci == 0), stop=(ci == CI_CH - 1),
                            )
                        et[t0] = ep.tile([K, MF], F32R, tag="e", name=f"e{t0}")
                        nc.scalar.activation(
                            et[t0], sim, mybir.ActivationFunctionType.Exp,
                            scale=SCALE, bias=sbias,
                        )
                    if 0 <= t1 < NMT:
                        # S1: denom + reciprocal
                        dent[t1] = denps.tile([1, MF], F32, tag="den", name=f"den{t1}")
                        nc.tensor.matmul(
                            dent[t1], ones_col, et[t1], start=True, stop=True,
                        )
                        rt[t1] = rp.tile([1, MF], F32R, tag="r", name=f"r{t1}")
                        with nc.allow_low_precision(reason="f32r is 4-byte fp32"):
                            nc.vector.reciprocal(rt[t1], dent[t1])
                        del dent[t1]
                    if 0 <= t2 < NMT:
                        # S2: broadcast recip over k, normalize
                        rbt[t2] = rbps.tile([K, MF], F32, tag="rb", name=f"rb{t2}")
                        nc.tensor.matmul(
                            rbt[t2], ones_row[:, :K], rt[t2],
                            start=True, stop=True,
                        )
                        ent[t2] = enp.tile([K, MF], F32R, tag="en", name=f"en{t2}")
                        nc.vector.tensor_tensor(
                            ent[t2], rbt[t2], et[t2], mybir.AluOpType.mult,
                        )
                        del rbt[t2], rt[t2], et[t2], xt[t2]
                    if 0 <= t3 < NMT:
                        # S3: out matmuls directly in the u8 domain (scale +
                        # bias folded into wvtq), pure f32->u8 copies, DMA
                        ps_a = outaps.tile([P, 2, MF], F32, tag="psa")
                        ps_b = outbps.tile([P, 2, MF], F32, tag="psb")
                        for oi in range(O_CH):
                            dst = ps_a if oi < 2 else ps_b
                            nc.tensor.matmul(
                                dst[:, oi % 2, :],
                                wvtq[:, oi * P:(oi + 1) * P],
                                ent[t3],
                                start=True, stop=True,
                            )
                        u8 = outp.tile([P, O_CH, MF], U8, tag="out")
                        nc.scalar.activation(
                            u8[:, 0:2, :], ps_a,
                            mybir.ActivationFunctionType.Identity,
                        )
                        nc.vector.tensor_copy(u8[:, 2, :], ps_b[:, 0, :])
                        nc.scalar.activation(
                            u8[:, 3, 0:416], ps_b[:, 1, 0:416],
                            mybir.ActivationFunctionType.Identity,
                        )
                        nc.vector.tensor_copy(
                            u8[:, 3, 416:MF], ps_b[:, 1, 416:MF],
                        )
                        nc.sync.dma_start(
                            out_r[:, :, t3 * MF:(t3 + 1) * MF], u8,
                        )
                        del ent[t3]

    nc.compile()
    return nc


def _get_nc():
    global _CACHED
    if _CACHED is None:
        _CACHED = _build()
    return _CACHED


def kernel(x, proxy, Wq, bq, Wk, bk, Wv, bv, Wo, bo, **run_kwargs):
    nc = _get_nc()

    crow = np.concatenate(
        [np.asarray(bk, np.float32).reshape(1, CK),
         np.asarray(bv, np.float32).reshape(1, CV),
         np.ones((1, 256), np.float32),
         np.asarray(bo, np.float32).reshape(1, C)], axis=1)
    bqbo = np.concatenate(
        [np.asarray(bq, np.float32).reshape(2, P).T,
         np.asarray(bo, np.float32).reshape(4, P).T], axis=1)
    w16 = np.concatenate(
        [np.asarray(Wk).T, np.asarray(Wv).T], axis=1
    ).astype(np.float16)
    shared = {
        "woT": np.ascontiguousarray(Wo.T).astype(np.float16),
        "wq": np.ascontiguousarray(Wq).astype(np.float16),
        "bqs16": np.ascontiguousarray(
            (np.asarray(bq, np.float32) * SCALE).reshape(2, P).T
        ).astype(np.float16),
        "crow": np.ascontiguousarray(crow),
        "bqbo": np.ascontiguousarray(bqbo),
        "ones_col": np.ones((K, 1), np.float32),
    }
    in_maps = []
    for b in range(B):
        m = dict(shared)
        m["x"] = np.ascontiguousarray(x[b]).reshape(C, N).astype(np.float16)
        pf16 = np.asarray(proxy[b, :, :, 0]).astype(np.float16)
        m["pack16"] = np.ascontiguousarray(np.concatenate([pf16, w16], axis=1))
        in_maps.append(m)

    res = bass_utils.run_bass_kernel_spmd(
        nc, in_maps, core_ids=list(range(B)), **run_kwargs
    )
    kernel.last_results = res
    out = np.empty((B, C, N), np.float32)
    for b in range(B):
        u8 = res.results[b]["out"].astype(np.float32)
        sinv = np.asarray(res.results[b]["sinv"], np.float32).reshape(C)
        out[b] = (u8 - 128.0) * sinv[:, None]
    return out.reshape(B, C, H, W)


# revision 18
# speedup vs baseline: 1.1014x; 1.0234x over previous
"""ObjectAttentionBlock2D TRN2 kernel.

Reference computation (per batch b):
    xf    = x[b].reshape(C, N)                  # C=512, N=128*128=16384
    pf    = proxy[b,:,:,0]                      # [C, K], K=64
    query = Wq @ xf + bq                        # [Ck=256, N]
    keym  = Wk @ pf + bk                        # [Ck, K]
    value = (Wv @ pf + bv).T                    # [K, Cv=256]
    sim   = softmax_k(query.T @ keym / 16)      # [N, K]
    ctx   = sim @ value                         # [N, Cv]
    out   = Wo @ ctx.T + bo                     # [C, N]

Sharding: data-parallel over batch. B=8 batches -> 8 NeuronCores, one image
per core, no collectives. Weights are replicated (host pre-transposes them so
the contraction dim is the SBUF partition dim).

Key algebraic optimization: the attention-logit and output maps are both
rank-K (K=64), and query/ctx each feed exactly one matmul, so both
projections fold into small precomputed matrices (on-device, per core):
  M     = Wq^T @ keym            [C, K]   -> simT = M^T x (4 MMs, was 10)
  sbias = (bq/16)^T @ keym       [K, 1]   -> rides in exp's bias slot
  WVT   = (Wo @ value^T)^T       [K, C]   -> out = WVT^T expPn (4 MMs, was 10)

The kernel is DMA-bound (in the TimelineSim cost model every DMA serializes
on one 360 GB/s DMA-engine pool), so both HBM streams are compressed:
  - x  is fp16 (halves the input stream; 10-bit mantissa suffices; fp8
    x was measured at 1.7e-2 end-to-end error vs the 2e-2 gate - too
    close, because logit quantization noise is amplified by softmax).
  - out is uint8 with an exact per-channel scale: out[c,:] is a convex
    combination (softmax weights) of WVT[:,c] entries plus bo[c], so
    bound_c = max_k |WVT[k,c] + bo[c]| bounds |out[c,:]| EXACTLY. The
    device computes bound_c (8 extra tiny matmuls give WVT^T in the
    partition layout of the output, then a free-axis abs-max), quantizes
    u8 = s_c*out + s_c*bo + 128 in the output-stage scale/bias slots
    (s_c = 126.5/bound_c), and ships sinv_c = bound_c/126.5 back; the
    host dequantizes (u8 - 128) * sinv. HW converts f32->u8 with
    round-to-nearest (verified empirically: offset 128.0 beats 127.5/
    128.5 2x). Output quantization adds only bounded absolute error (no
    softmax amplification): measured end-to-end rel err ~6.5e-3.

Engine-cost model (TimelineSim): every ACT/DVE instruction costs
~125-185ns fixed + ~1ns/free-elem, and a matmul costs out_free_size x
0.44ns regardless of contraction rows. At F=256 the per-instruction
fixed costs made DVE (104us) and ACT (78us) the critical path, so the
pipeline runs F=512 tiles: per 512-px tile 10 MMs, 1 exp, 1 recip, 1
en-mult, and the 4 quantize chunks are spread ACT/ACT/DVE/Pool to
balance engines. Predicted busy: DMA 74us > PE ~69 > DVE ~63 > Pool
~60 > ACT ~59 -> DMA-bound again.

Per-core pipeline over 32 macro-tiles of MF=512 pixels:
  x DMA [128, 4, 512] fp16 on gpsimd/SWDGE (1024B descriptors; >=512B
    dodges the sub-512B read-modify-write 2x penalty, and one dispatch
    per 512 px halves SWDGE descriptor-generation time on Pool)
  simT [K=64, 512] (4 fp16 MMs, contract C=512, lhsT=M)
  ACT exp(sim/16 + sbias) -> f32r SBUF
  denom = ones64^T @ expP (1 MM) -> DVE reciprocal -> K=1 broadcast MM
  expPn = expP * recip (DVE)
  out [C, 512] (4 f32r MMs, contract K=64, lhsT=WVT) -> quantize 4
    chunks (ACT/ACT/DVE/Pool) -> u8 SBUF -> one DMA on the SP HWDGE
    queue (512B descriptors)
PSUM: sim 2 bufs x 1 bank + den 1 + rb 1 + outps 1 buf x 4 banks = 8.
keym/V2/M/sbias/WVT/bounds are precomputed once per core with biases
folded in via K=1 matmul accumulation (bias outer-product with ones).

Precision: x/Wq/Wk/Wv/Wo/pf are cast to fp16 on the host; everything
downstream runs float32r (1 cycle/row at free>=256) with fp32 PSUM
accumulation. Output u8 + per-channel scale as above.
"""

import numpy as np

import concourse.bacc as bacc
import concourse.mybir as mybir
import concourse.tile as tile
from concourse import bass_utils

F32 = mybir.dt.float32
F32R = mybir.dt.float32r
F16 = mybir.dt.float16
U8 = mybir.dt.uint8

B, C, H, W = 8, 512, 128, 128
N = H * W                    # 16384 pixels per image
CK, CV, K = 256, 256, 64
P = 128                      # SBUF partitions
MF = 512                     # pixel tile width (compute + DMA)
NMT = N // MF                # 32 tiles
CI_CH = C // P               # 4 contraction chunks over C
Q_CH = CK // P               # 2 chunks over Ck
V_CH = CV // P               # 2 chunks over Cv
O_CH = C // P                # 4 chunks over output C
SCALE = CK ** -0.5           # 1/16
QCAP = 126.5                 # |s*out| <= 126.5 so u8 = s*out+128 in [1.5, 254.5]

_CACHED = None


def _build():
    nc = bacc.Bacc("TRN2", target_bir_lowering=False, debug=False)

    X = nc.dram_tensor("x", [C, N], F16, kind="ExternalInput").ap()
    # pack16[c, :] = [pf(64) | wkT(256) | wvT(256)] in fp16
    PACK16 = nc.dram_tensor("pack16", [C, 576], F16, kind="ExternalInput").ap()
    WQ = nc.dram_tensor("wq", [CK, C], F16, kind="ExternalInput").ap()
    # crow = [bk(256) | bv(256) | ones(256) | bo(512)] as one row
    CROW = nc.dram_tensor("crow", [1, 1280], F32, kind="ExternalInput").ap()
    ONESC = nc.dram_tensor("ones_col", [K, 1], F32, kind="ExternalInput").ap()
    # bqbo[p, :] = [bq 2 chunks | bo 4 chunks] per-partition layout
    BQBO = nc.dram_tensor("bqbo", [P, 6], F32, kind="ExternalInput").ap()
    BQS16 = nc.dram_tensor("bqs16", [P, 2], F16, kind="ExternalInput").ap()
    WOT = nc.dram_tensor("woT", [CV, C], F16, kind="ExternalInput").ap()
    OUT = nc.dram_tensor("out", [C, N], U8, kind="ExternalOutput").ap()
    SINV = nc.dram_tensor("sinv", [1, C], F32, kind="ExternalOutput").ap()

    x_r = X.rearrange("(co p) n -> p co n", p=P)       # [128, 4, N]
    out_r = OUT.rearrange("(oo p) n -> p oo n", p=P)   # [128, 4, N]

    with tile.TileContext(nc) as tc:
        with tc.tile_pool(name="const", bufs=1) as cp:
            pack = cp.tile([P, CI_CH, 576], F16)
            nc.sync.dma_start(pack, PACK16.rearrange("(co p) q -> p co q", p=P))
            pf = pack[:, :, 0:K]
            wk = pack[:, :, K:K + CK]
            wv = pack[:, :, K + CK:K + CK + CV]
            wq = cp.tile([P, Q_CH, C], F16)
            nc.sync.dma_start(wq, WQ.rearrange("(qo p) c -> p qo c", p=P))
            crow = cp.tile([1, 1280], F32R)
            nc.sync.dma_start(crow, CROW.bitcast(F32R))
            bk_row = crow[:, 0:CK]
            bv_row = crow[:, CK:CK + CV]
            ones_row = crow[:, 512:768]
            bo_row = crow[:, 768:1280]
            ones_col = cp.tile([K, 1], F32R)
            nc.sync.dma_start(ones_col, ONESC.bitcast(F32R))
            bqbo = cp.tile([P, 6], F32)
            nc.scalar.dma_start(bqbo, BQBO)
            bqs16 = cp.tile([P, 2], F16)
            nc.scalar.dma_start(bqs16, BQS16)
            bqs = bqs16
            bo = bqbo[:, 2:6]
            wo = cp.tile([P, V_CH, C], F16)
            nc.scalar.dma_start(wo, WOT.rearrange("(vo p) o -> p vo o", p=P))

            keym = cp.tile([P, Q_CH, K], F16)    # [q-part, q-chunk, k]
            msim = cp.tile([P, CI_CH, K], F16)   # M[c,k] = sum_q Wq[q,c]*keym[q,k]
            sbias = cp.tile([K, 1], F32)         # sum_q (bq[q]/16)*keym[q,k]
            # u8 quantization: out[c,:] is a convex combination (softmax
            # weights sum to 1) of WVT[:,c]+bo[c], so with
            #   wvtq[k,c] = s_c*(WVT[k,c]+bo[c]) + 128,  s_c = 126.5/bound_c
            # the out matmul directly yields u8-domain values in [1.5,254.5]
            # (the +128 rides on sum_k en = 1) and the output stage is a pure
            # f32->u8 copy with no per-channel scalars.
            wvtb = cp.tile([K, C], F32R)         # WVT[k,c] + bo[c]
            wvtq = cp.tile([K, C], F32R)         # s_c*(WVT+bo) + 128
            bound_row = cp.tile([1, C], F32R)    # max_k |WVT[k,c]+bo[c]|
            r_row = cp.tile([1, C], F32R)
            s_row = cp.tile([1, C], F32R)        # 126.5 / bound
            sinv_row = cp.tile([1, C], F32R)     # bound / 126.5

            # ---- one-time: keym = Wk @ pf + bk, value[k,v] = (Wv @ pf + bv)[v,k]
            with tc.tile_pool(name="setup_ps", bufs=1, space="PSUM") as sps:
                kps = sps.tile([P, Q_CH, K], F32)
                for qi in range(Q_CH):
                    for ci in range(CI_CH):
                        nc.tensor.matmul(
                            kps[:, qi, :],
                            wk[:, ci, qi * P:(qi + 1) * P],
                            pf[:, ci, :],
                            start=(ci == 0), stop=False,
                        )
                    # += bk[q] * ones[k]
                    nc.tensor.matmul(
                        kps[:, qi, :],
                        bk_row[:, qi * P:(qi + 1) * P],
                        ones_row[:, :K],
                        start=False, stop=True,
                    )
                nc.vector.tensor_copy(keym, kps)

                v2ps = sps.tile([P, V_CH, K], F32)
                for vi in range(V_CH):
                    for ci in range(CI_CH):
                        nc.tensor.matmul(
                            v2ps[:, vi, :],
                            wv[:, ci, vi * P:(vi + 1) * P],
                            pf[:, ci, :],
                            start=(ci == 0), stop=False,
                        )
                    nc.tensor.matmul(
                        v2ps[:, vi, :],
                        bv_row[:, vi * P:(vi + 1) * P],
                        ones_row[:, :K],
                        start=False, stop=True,
                    )
                v2sb = cp.tile([P, V_CH, K], F16)
                nc.vector.tensor_copy(v2sb, v2ps)
                wvtps = sps.tile([K, C], F32)
                for vi in range(V_CH):
                    nc.tensor.matmul(
                        wvtps, v2sb[:, vi, :], wo[:, vi, :],
                        start=(vi == 0), stop=False,
                    )
                # += bo broadcast along k (rank-1 accumulate) -> WVT + bo
                nc.tensor.matmul(
                    wvtps, ones_row[:, :K], bo_row, start=False, stop=True,
                )
                with nc.allow_low_precision(reason="f32r is 4-byte fp32"):
                    nc.vector.tensor_copy(wvtb, wvtps)
                    # exact |out| bound per channel: abs on DVE (gpsimd's
                    # apply_absolute_value is ignored on HW), then
                    # cross-partition max on gpsimd
                    wvta = cp.tile([K, C], F32R)
                    nc.vector.tensor_scalar(
                        wvta, wvtb, -1.0, None, op0=mybir.AluOpType.mult,
                    )
                    nc.vector.tensor_tensor(wvta, wvtb, wvta, mybir.AluOpType.max)
                    nc.gpsimd.tensor_reduce(
                        bound_row, wvta, axis=mybir.AxisListType.C,
                        op=mybir.AluOpType.max,
                    )
                    nc.vector.tensor_scalar(
                        bound_row, bound_row, 1e-3, None, op0=mybir.AluOpType.max,
                    )
                    nc.vector.reciprocal(r_row, bound_row)
                    nc.vector.tensor_scalar(
                        s_row, r_row, QCAP, None, op0=mybir.AluOpType.mult,
                    )
                    nc.vector.tensor_scalar(
                        sinv_row, bound_row, 1.0 / QCAP, None,
                        op0=mybir.AluOpType.mult,
                    )
                    nc.scalar.dma_start(SINV, sinv_row.bitcast(F32))
                    # wvtq = s*(WVT+bo) + 128, via rank-1 broadcast of s
                    s_bc = sps.tile([K, C], F32)
                    nc.tensor.matmul(s_bc, ones_row[:, :K], s_row, start=True, stop=True)
                    nc.vector.tensor_tensor(wvtq, wvtb, s_bc, mybir.AluOpType.mult)
                    nc.vector.tensor_scalar(
                        wvtq, wvtq, 128.0, None, op0=mybir.AluOpType.add,
                    )

                # M: fold the Q projection into the sim matmul (Q only feeds sim)
                mps = sps.tile([P, CI_CH, K], F32)
                for ci in range(CI_CH):
                    for qi in range(Q_CH):
                        nc.tensor.matmul(
                            mps[:, ci, :],
                            wq[:, qi, ci * P:(ci + 1) * P],
                            keym[:, qi, :],
                            start=(qi == 0), stop=(qi == Q_CH - 1),
                        )
                nc.vector.tensor_copy(msim, mps)
                # sbias[k,1]: lhsT=keym chunks, rhs=bq/16 column
                sbps = sps.tile([K, 1], F32)
                for qi in range(Q_CH):
                    nc.tensor.matmul(
                        sbps, keym[:, qi, :], bqs[:, qi:qi + 1],
                        start=(qi == 0), stop=(qi == Q_CH - 1),
                    )
                nc.vector.tensor_copy(sbias, sbps)

            # ---- steady-state: 4-stage software pipeline over 512-px tiles
            # Iteration i runs  S0: sim+exp(i) | S1: den+recip(i-1) |
            # S2: rb+en(i-2) | S3: out MMs + u8 copies + DMA(i-3).
            # The softmax chain has 3 cross-engine round trips; skewing the
            # stages gives every dependency a full iteration of slack so each
            # in-order engine queue never waits (the naive fused loop ran all
            # engines at ~50% on exactly those round trips).
            # Copy split tuned to balance ACT (exp + 1038 + 532 = 2182ns/it)
            # vs DVE (recip + en + 658 + 225 = 2199ns/it), both under the
            # 2287ns/it DMA floor.
            PRE = 2   # x-DMA prefetch distance (iterations)
            with (
                tc.tile_pool(name="xin", bufs=5) as xp,
                tc.tile_pool(name="esb", bufs=4) as ep,
                tc.tile_pool(name="rsb", bufs=3) as rp,
                tc.tile_pool(name="ensb", bufs=3) as enp,
                tc.tile_pool(name="outsb", bufs=3) as outp,
                tc.tile_pool(name="sdps", bufs=1, space="PSUM") as sdps,
                tc.tile_pool(name="denps", bufs=2, space="PSUM") as denps,
                tc.tile_pool(name="rbps", bufs=1, space="PSUM") as rbps,
                tc.tile_pool(name="outaps", bufs=1, space="PSUM") as outaps,
                tc.tile_pool(name="outbps", bufs=1, space="PSUM") as outbps,
            ):
                xt = {}
                et = {}
                rt = {}
                dent = {}
                rbt = {}
                ent = {}

                def fetch_x(t):
                    # HWDGE on alternating scalar/vector queues: the SWDGE
                    # ring holds only 1024 descriptors (= 2 of these DMAs),
                    # which serialized gen->transfer->gen at 3.3us/tile.
                    if t < NMT:
                        xt[t] = xp.tile([P, CI_CH, MF], F16, tag="x", name=f"x{t}")
                        eng = nc.scalar if t % 2 == 0 else nc.sync
                        eng.dma_start(xt[t], x_r[:, :, t * MF:(t + 1) * MF])

                for t in range(PRE):
                    fetch_x(t)

                for i in range(NMT + 3):
                    fetch_x(i + PRE)
                    t0, t1, t2, t3 = i, i - 1, i - 2, i - 3
                    if t0 < NMT:
                        # S0: simT[k,n] = M^T-contract-c @ x, then exp
                        sim = sdps.tile([K, MF], F32, tag="sd")
                        for ci in range(CI_CH):
                            nc.tensor.matmul(
                                sim, msim[:, ci, :], xt[t0][:, ci, :],
                                start=(ci == 0), stop=(ci == CI_CH - 1),
                            )
                        et[t0] = ep.tile([K, MF], F32R, tag="e", name=f"e{t0}")
                        nc.scalar.activation(
                            et[t0], sim, mybir.ActivationFunctionType.Exp,
                            scale=SCALE, bias=sbias,
                        )
                    if 0 <= t1 < NMT:
                        # S1: denom + reciprocal
                        dent[t1] = denps.tile([1, MF], F32, tag="den", name=f"den{t1}")
                        nc.tensor.matmul(
                            dent[t1], ones_col, et[t1], start=True, stop=True,
                        )
                        rt[t1] = rp.tile([1, MF], F32R, tag="r", name=f"r{t1}")
                        with nc.allow_low_precision(reason="f32r is 4-byte fp32"):
                            nc.vector.reciprocal(rt[t1], dent[t1])
                        del dent[t1]
                    if 0 <= t2 < NMT:
                        # S2: broadcast recip over k, normalize
                        rbt[t2] = rbps.tile([K, MF], F32, tag="rb", name=f"rb{t2}")
                        nc.tensor.matmul(
                            rbt[t2], ones_row[:, :K], rt[t2],
                            start=True, stop=True,
                        )
                        ent[t2] = enp.tile([K, MF], F32R, tag="en", name=f"en{t2}")
                        nc.vector.tensor_tensor(
                            ent[t2], rbt[t2], et[t2], mybir.AluOpType.mult,
                        )
                        del rbt[t2], rt[t2], et[t2], xt[t2]
                    if 0 <= t3 < NMT:
                        # S3: out matmuls directly in the u8 domain (scale +
                        # bias folded into wvtq), pure f32->u8 copies, DMA
                        ps_a = outaps.tile([P, 2, MF], F32, tag="psa")
                        ps_b = outbps.tile([P, 2, MF], F32, tag="psb")
                        for oi in range(O_CH):
                            dst = ps_a if oi < 2 else ps_b
                            nc.tensor.matmul(
                                dst[:, oi % 2, :],
                                wvtq[:, oi * P:(oi + 1) * P],
                                ent[t3],
                                start=True, stop=True,
                            )
                        u8 = outp.tile([P, O_CH, MF], U8, tag="out")
                        nc.scalar.activation(
                            u8[:, 0:2, :], ps_a,
                            mybir.ActivationFunctionType.Identity,
                        )
                        nc.vector.tensor_copy(u8[:, 2, :], ps_b[:, 0, :])
                        nc.scalar.activation(
                            u8[:, 3, 0:416], ps_b[:, 1, 0:416],
                            mybir.ActivationFunctionType.Identity,
                        )
                        nc.vector.tensor_copy(
                            u8[:, 3, 416:MF], ps_b[:, 1, 416:MF],
                        )
                        nc.sync.dma_start(
                            out_r[:, :, t3 * MF:(t3 + 1) * MF], u8,
                        )
                        del ent[t3]

    nc.compile()
    return nc


def _get_nc():
    global _CACHED
    if _CACHED is None:
        _CACHED = _build()
    return _CACHED


def kernel(x, proxy, Wq, bq, Wk, bk, Wv, bv, Wo, bo, **run_kwargs):
    nc = _get_nc()

    crow = np.concatenate(
        [np.asarray(bk, np.float32).reshape(1, CK),
         np.asarray(bv, np.float32).reshape(1, CV),
         np.ones((1, 256), np.float32),
         np.asarray(bo, np.float32).reshape(1, C)], axis=1)
    bqbo = np.concatenate(
        [np.asarray(bq, np.float32).reshape(2, P).T,
         np.asarray(bo, np.float32).reshape(4, P).T], axis=1)
    w16 = np.concatenate(
        [np.asarray(Wk).T, np.asarray(Wv).T], axis=1
    ).astype(np.float16)
    shared = {
        "woT": np.ascontiguousarray(Wo.T).astype(np.float16),
        "wq": np.ascontiguousarray(Wq).astype(np.float16),
        "bqs16": np.ascontiguousarray(
            (np.asarray(bq, np.float32) * SCALE).reshape(2, P).T
        ).astype(np.float16),
        "crow": np.ascontiguousarray(crow),
        "bqbo": np.ascontiguousarray(bqbo),
        "ones_col": np.ones((K, 1), np.float32),
    }
    in_maps = []
    for b in range(B):
        m = dict(shared)
        m["x"] = np.ascontiguousarray(x[b]).reshape(C, N).astype(np.float16)
        pf16 = np.asarray(proxy[b, :, :, 0]).astype(np.float16)
        m["pack16"] = np.ascontiguousarray(np.concatenate([pf16, w16], axis=1))
        in_maps.append(m)

    res = bass_utils.run_bass_kernel_spmd(
        nc, in_maps, core_ids=list(range(B)), **run_kwargs
    )
    kernel.last_results = res
    out = np.empty((B, C, N), np.float32)
    for b in range(B):
        u8 = res.results[b]["out"].astype(np.float32)
        sinv = np.asarray(res.results[b]["sinv"], np.float32).reshape(C)
        out[b] = (u8 - 128.0) * sinv[:, None]
    return out.reshape(B, C, H, W)


# revision 20
# speedup vs baseline: 1.3092x; 1.1887x over previous
"""ObjectAttentionBlock2D TRN2 kernel.

Reference computation (per batch b):
    xf    = x[b].reshape(C, N)                  # C=512, N=128*128=16384
    pf    = proxy[b,:,:,0]                      # [C, K], K=64
    query = Wq @ xf + bq                        # [Ck=256, N]
    keym  = Wk @ pf + bk                        # [Ck, K]
    value = (Wv @ pf + bv).T                    # [K, Cv=256]
    sim   = softmax_k(query.T @ keym / 16)      # [N, K]
    ctx   = sim @ value                         # [N, Cv]
    out   = Wo @ ctx.T + bo                     # [C, N]

Sharding: data-parallel over batch. B=8 batches -> 8 NeuronCores, one image
per core, no collectives. Weights are replicated (host pre-transposes them so
the contraction dim is the SBUF partition dim).

Key algebraic optimization: the attention-logit and output maps are both
rank-K (K=64), and query/ctx each feed exactly one matmul, so both
projections fold into small precomputed matrices (on-device, per core):
  M     = Wq^T @ keym            [C, K]   -> simT = M^T x (4 MMs, was 10)
  sbias = (bq/16)^T @ keym       [K, 1]   -> rides in exp's bias slot
  WVT   = (Wo @ value^T)^T       [K, C]   -> out = WVT^T expPn (4 MMs, was 10)

The kernel is DMA-bound (in the TimelineSim cost model every DMA serializes
on one 360 GB/s DMA-engine pool), so both HBM streams are compressed:
  - x  is fp16 (halves the input stream; 10-bit mantissa suffices; fp8
    x was measured at 1.7e-2 end-to-end error vs the 2e-2 gate - too
    close, because logit quantization noise is amplified by softmax).
  - out is uint8 with an exact per-channel scale: out[c,:] is a convex
    combination (softmax weights) of WVT[:,c] entries plus bo[c], so
    bound_c = max_k |WVT[k,c] + bo[c]| bounds |out[c,:]| EXACTLY. The
    device computes bound_c (8 extra tiny matmuls give WVT^T in the
    partition layout of the output, then a free-axis abs-max), quantizes
    u8 = s_c*out + s_c*bo + 128 in the output-stage scale/bias slots
    (s_c = 126.5/bound_c), and ships sinv_c = bound_c/126.5 back; the
    host dequantizes (u8 - 128) * sinv. HW converts f32->u8 with
    round-to-nearest (verified empirically: offset 128.0 beats 127.5/
    128.5 2x). Output quantization adds only bounded absolute error (no
    softmax amplification): measured end-to-end rel err ~6.5e-3.

Engine-cost model (TimelineSim): every ACT/DVE instruction costs
~125-185ns fixed + ~1ns/free-elem, and a matmul costs out_free_size x
0.44ns regardless of contraction rows. At F=256 the per-instruction
fixed costs made DVE (104us) and ACT (78us) the critical path, so the
pipeline runs F=512 tiles: per 512-px tile 10 MMs, 1 exp, 1 recip, 1
en-mult, and the 4 quantize chunks are spread ACT/ACT/DVE/Pool to
balance engines. Predicted busy: DMA 74us > PE ~69 > DVE ~63 > Pool
~60 > ACT ~59 -> DMA-bound again.

Per-core pipeline over 32 macro-tiles of MF=512 pixels:
  x DMA [128, 4, 512] fp16 on gpsimd/SWDGE (1024B descriptors; >=512B
    dodges the sub-512B read-modify-write 2x penalty, and one dispatch
    per 512 px halves SWDGE descriptor-generation time on Pool)
  simT [K=64, 512] (4 fp16 MMs, contract C=512, lhsT=M)
  ACT exp(sim/16 + sbias) -> f32r SBUF
  denom = ones64^T @ expP (1 MM) -> DVE reciprocal -> K=1 broadcast MM
  expPn = expP * recip (DVE)
  out [C, 512] (4 f32r MMs, contract K=64, lhsT=WVT) -> quantize 4
    chunks (ACT/ACT/DVE/Pool) -> u8 SBUF -> one DMA on the SP HWDGE
    queue (512B descriptors)
PSUM: sim 2 bufs x 1 bank + den 1 + rb 1 + outps 1 buf x 4 banks = 8.
keym/V2/M/sbias/WVT/bounds are precomputed once per core with biases
folded in via K=1 matmul accumulation (bias outer-product with ones).

Precision: x/Wq/Wk/Wv/Wo/pf are cast to fp16 on the host; everything
downstream runs float32r (1 cycle/row at free>=256) with fp32 PSUM
accumulation. Output u8 + per-channel scale as above.
"""

import numpy as np

import concourse.bacc as bacc
import concourse.mybir as mybir
import concourse.tile as tile
from concourse import bass_utils

F32 = mybir.dt.float32
F32R = mybir.dt.float32r
F16 = mybir.dt.float16
U8 = mybir.dt.uint8

B, C, H, W = 8, 512, 128, 128
N = H * W                    # 16384 pixels per image
CK, CV, K = 256, 256, 64
P = 128                      # SBUF partitions
MF = 512                     # pixel tile width (compute + DMA)
NMT = N // MF                # 32 tiles
CI_CH = C // P               # 4 contraction chunks over C
Q_CH = CK // P               # 2 chunks over Ck
V_CH = CV // P               # 2 chunks over Cv
O_CH = C // P                # 4 chunks over output C
SCALE = CK ** -0.5           # 1/16
QCAP = 126.5                 # |s*out| <= 126.5 so u8 = s*out+128 in [1.5, 254.5]

_CACHED = None


def _build():
    nc = bacc.Bacc("TRN2", target_bir_lowering=False, debug=False)

    X = nc.dram_tensor("x", [C, N], F16, kind="ExternalInput").ap()
    # pack16[c, :] = [pf(64) | wkT(256) | wvT(256)] in fp16
    PACK16 = nc.dram_tensor("pack16", [C, 576], F16, kind="ExternalInput").ap()
    WQ = nc.dram_tensor("wq", [CK, C], F16, kind="ExternalInput").ap()
    # crow = [bk(256) | bv(256) | ones(256) | bo(512)] as one row
    CROW = nc.dram_tensor("crow", [1, 1280], F32, kind="ExternalInput").ap()
    ONESC = nc.dram_tensor("ones_col", [K, 1], F32, kind="ExternalInput").ap()
    # bqbo[p, :] = [bq 2 chunks | bo 4 chunks] per-partition layout
    BQBO = nc.dram_tensor("bqbo", [P, 6], F32, kind="ExternalInput").ap()
    BQS16 = nc.dram_tensor("bqs16", [P, 2], F16, kind="ExternalInput").ap()
    WOT = nc.dram_tensor("woT", [CV, C], F16, kind="ExternalInput").ap()
    OUT = nc.dram_tensor("out", [C, N], U8, kind="ExternalOutput").ap()
    SINV = nc.dram_tensor("sinv", [1, C], F32, kind="ExternalOutput").ap()

    x_r = X.rearrange("(co p) n -> p co n", p=P)       # [128, 4, N]
    out_r = OUT.rearrange("(oo p) n -> p oo n", p=P)   # [128, 4, N]

    with tile.TileContext(nc) as tc:
        with tc.tile_pool(name="const", bufs=1) as cp:
            pack = cp.tile([P, CI_CH, 576], F16)
            nc.sync.dma_start(pack, PACK16.rearrange("(co p) q -> p co q", p=P))
            pf = pack[:, :, 0:K]
            wk = pack[:, :, K:K + CK]
            wv = pack[:, :, K + CK:K + CK + CV]
            wq = cp.tile([P, Q_CH, C], F16)
            nc.sync.dma_start(wq, WQ.rearrange("(qo p) c -> p qo c", p=P))
            crow = cp.tile([1, 1280], F32R)
            nc.sync.dma_start(crow, CROW.bitcast(F32R))
            bk_row = crow[:, 0:CK]
            bv_row = crow[:, CK:CK + CV]
            ones_row = crow[:, 512:768]
            bo_row = crow[:, 768:1280]
            ones_col = cp.tile([K, 1], F32R)
            nc.sync.dma_start(ones_col, ONESC.bitcast(F32R))
            bqbo = cp.tile([P, 6], F32)
            nc.scalar.dma_start(bqbo, BQBO)
            bqs16 = cp.tile([P, 2], F16)
            nc.scalar.dma_start(bqs16, BQS16)
            bqs = bqs16
            bo = bqbo[:, 2:6]
            wo = cp.tile([P, V_CH, C], F16)
            nc.scalar.dma_start(wo, WOT.rearrange("(vo p) o -> p vo o", p=P))

            keym = cp.tile([P, Q_CH, K], F16)    # [q-part, q-chunk, k]
            msim = cp.tile([P, CI_CH, K], F16)   # M[c,k] = sum_q Wq[q,c]*keym[q,k]
            sbias = cp.tile([K, 1], F32)         # sum_q (bq[q]/16)*keym[q,k]
            # u8 quantization: out[c,:] is a convex combination (softmax
            # weights sum to 1) of WVT[:,c]+bo[c], so with
            #   wvtq[k,c] = s_c*(WVT[k,c]+bo[c]) + 128,  s_c = 126.5/bound_c
            # the out matmul directly yields u8-domain values in [1.5,254.5]
            # (the +128 rides on sum_k en = 1) and the output stage is a pure
            # f32->u8 copy with no per-channel scalars.
            wvtb = cp.tile([K, C], F32R)         # WVT[k,c] + bo[c]
            wvtq = cp.tile([K, C], F32R)         # s_c*(WVT+bo) + 128
            bound_row = cp.tile([1, C], F32R)    # max_k |WVT[k,c]+bo[c]|
            r_row = cp.tile([1, C], F32R)
            s_row = cp.tile([1, C], F32R)        # 126.5 / bound
            sinv_row = cp.tile([1, C], F32R)     # bound / 126.5

            # ---- one-time: keym = Wk @ pf + bk, value[k,v] = (Wv @ pf + bv)[v,k]
            with tc.tile_pool(name="setup_ps", bufs=1, space="PSUM") as sps:
                kps = sps.tile([P, Q_CH, K], F32)
                for qi in range(Q_CH):
                    for ci in range(CI_CH):
                        nc.tensor.matmul(
                            kps[:, qi, :],
                            wk[:, ci, qi * P:(qi + 1) * P],
                            pf[:, ci, :],
                            start=(ci == 0), stop=False,
                        )
                    # += bk[q] * ones[k]
                    nc.tensor.matmul(
                        kps[:, qi, :],
                        bk_row[:, qi * P:(qi + 1) * P],
                        ones_row[:, :K],
                        start=False, stop=True,
                    )
                nc.vector.tensor_copy(keym, kps)

                v2ps = sps.tile([P, V_CH, K], F32)
                for vi in range(V_CH):
                    for ci in range(CI_CH):
                        nc.tensor.matmul(
                            v2ps[:, vi, :],
                            wv[:, ci, vi * P:(vi + 1) * P],
                            pf[:, ci, :],
                            start=(ci == 0), stop=False,
                        )
                    nc.tensor.matmul(
                        v2ps[:, vi, :],
                        bv_row[:, vi * P:(vi + 1) * P],
                        ones_row[:, :K],
                        start=False, stop=True,
                    )
                v2sb = cp.tile([P, V_CH, K], F16)
                nc.vector.tensor_copy(v2sb, v2ps)
                wvtps = sps.tile([K, C], F32)
                for vi in range(V_CH):
                    nc.tensor.matmul(
                        wvtps, v2sb[:, vi, :], wo[:, vi, :],
                        start=(vi == 0), stop=False,
                    )
                # += bo broadcast along k (rank-1 accumulate) -> WVT + bo
                nc.tensor.matmul(
                    wvtps, ones_row[:, :K], bo_row, start=False, stop=True,
                )
                with nc.allow_low_precision(reason="f32r is 4-byte fp32"):
                    nc.vector.tensor_copy(wvtb, wvtps)
                    # exact |out| bound per channel: abs on DVE (gpsimd's
                    # apply_absolute_value is ignored on HW), then
                    # cross-partition max on gpsimd
                    wvta = cp.tile([K, C], F32R)
                    nc.vector.tensor_scalar(
                        wvta, wvtb, -1.0, None, op0=mybir.AluOpType.mult,
                    )
                    nc.vector.tensor_tensor(wvta, wvtb, wvta, mybir.AluOpType.max)
                    nc.gpsimd.tensor_reduce(
                        bound_row, wvta, axis=mybir.AxisListType.C,
                        op=mybir.AluOpType.max,
                    )
                    nc.vector.tensor_scalar(
                        bound_row, bound_row, 1e-3, None, op0=mybir.AluOpType.max,
                    )
                    nc.vector.reciprocal(r_row, bound_row)
                    nc.vector.tensor_scalar(
                        s_row, r_row, QCAP, None, op0=mybir.AluOpType.mult,
                    )
                    nc.vector.tensor_scalar(
                        sinv_row, bound_row, 1.0 / QCAP, None,
                        op0=mybir.AluOpType.mult,
                    )
                    nc.scalar.dma_start(SINV, sinv_row.bitcast(F32))
                    # wvtq = s*(WVT+bo) + 128, via rank-1 broadcast of s
                    s_bc = sps.tile([K, C], F32)
                    nc.tensor.matmul(s_bc, ones_row[:, :K], s_row, start=True, stop=True)
                    nc.vector.tensor_tensor(wvtq, wvtb, s_bc, mybir.AluOpType.mult)
                    nc.vector.tensor_scalar(
                        wvtq, wvtq, 128.0, None, op0=mybir.AluOpType.add,
                    )

                # M: fold the Q projection into the sim matmul (Q only feeds sim)
                mps = sps.tile([P, CI_CH, K], F32)
                for ci in range(CI_CH):
                    for qi in range(Q_CH):
                        nc.tensor.matmul(
                            mps[:, ci, :],
                            wq[:, qi, ci * P:(ci + 1) * P],
                            keym[:, qi, :],
                            start=(qi == 0), stop=(qi == Q_CH - 1),
                        )
                nc.vector.tensor_copy(msim, mps)
                # sbias[k,1]: lhsT=keym chunks, rhs=bq/16 column
                sbps = sps.tile([K, 1], F32)
                for qi in range(Q_CH):
                    nc.tensor.matmul(
                        sbps, keym[:, qi, :], bqs[:, qi:qi + 1],
                        start=(qi == 0), stop=(qi == Q_CH - 1),
                    )
                nc.vector.tensor_copy(sbias, sbps)

            # ---- steady-state: 4-stage software pipeline over 512-px tiles
            # Iteration i runs  S0: sim+exp(i) | S1: den+recip(i-1) |
            # S2: rb+en(i-2) | S3: out MMs + u8 copies + DMA(i-3).
            # The softmax chain has 3 cross-engine round trips; skewing the
            # stages gives every dependency a full iteration of slack so each
            # in-order engine queue never waits (the naive fused loop ran all
            # engines at ~50% on exactly those round trips).
            # Copy split tuned to balance ACT (exp + 1038 + 532 = 2182ns/it)
            # vs DVE (recip + en + 658 + 225 = 2199ns/it), both under the
            # 2287ns/it DMA floor.
            PRE = 2   # x-DMA prefetch distance (iterations)
            with (
                tc.tile_pool(name="xin", bufs=5) as xp,
                tc.tile_pool(name="esb", bufs=4) as ep,
                tc.tile_pool(name="densb", bufs=2) as dnp,
                tc.tile_pool(name="rsb", bufs=3) as rp,
                tc.tile_pool(name="ensb", bufs=3) as enp,
                tc.tile_pool(name="outsb", bufs=3) as outp,
                tc.tile_pool(name="sdps", bufs=1, space="PSUM") as sdps,
                tc.tile_pool(name="rbps", bufs=1, space="PSUM") as rbps,
                tc.tile_pool(name="outaps", bufs=1, space="PSUM") as outaps,
                tc.tile_pool(name="outbps", bufs=2, space="PSUM") as outbps,
            ):
                xt = {}
                et = {}
                rt = {}
                dent = {}
                rbt = {}
                ent = {}

                def fetch_x(t):
                    # HWDGE on alternating scalar/vector queues: the SWDGE
                    # ring holds only 1024 descriptors (= 2 of these DMAs),
                    # which serialized gen->transfer->gen at 3.3us/tile.
                    if t < NMT:
                        xt[t] = xp.tile([P, CI_CH, MF], F16, tag="x", name=f"x{t}")
                        eng = nc.scalar if t % 2 == 0 else nc.sync
                        eng.dma_start(xt[t], x_r[:, :, t * MF:(t + 1) * MF])

                for t in range(PRE):
                    fetch_x(t)

                for i in range(NMT + 3):
                    fetch_x(i + PRE)
                    t0, t1, t2, t3 = i, i - 1, i - 2, i - 3
                    if t0 < NMT:
                        # S0: simT[k,n] = M^T-contract-c @ x, then exp
                        sim = sdps.tile([K, MF], F32, tag="sd")
                        for ci in range(CI_CH):
                            nc.tensor.matmul(
                                sim, msim[:, ci, :], xt[t0][:, ci, :],
                                start=(ci == 0), stop=(ci == CI_CH - 1),
                            )
                        et[t0] = ep.tile([K, MF], F32R, tag="e", name=f"e{t0}")
                        nc.scalar.activation(
                            et[t0], sim, mybir.ActivationFunctionType.Exp,
                            scale=SCALE, bias=sbias,
                        )
                    if 0 <= t1 < NMT:
                        # S1: denom on the otherwise-idle Pool engine (cross-
                        # partition sum, SBUF->SBUF — keeps PSUM banks and the
                        # PE free), then reciprocal on DVE
                        dent[t1] = dnp.tile([1, MF], F32R, tag="den", name=f"den{t1}")
                        with nc.allow_low_precision(reason="f32r is 4-byte fp32"):
                            nc.gpsimd.tensor_reduce(
                                dent[t1], et[t1], axis=mybir.AxisListType.C,
                                op=mybir.AluOpType.add,
                            )
                        rt[t1] = rp.tile([1, MF], F32R, tag="r", name=f"r{t1}")
                        with nc.allow_low_precision(reason="f32r is 4-byte fp32"):
                            nc.vector.reciprocal(rt[t1], dent[t1])
                        del dent[t1]
                    if 0 <= t2 < NMT:
                        # S2: broadcast recip over k, normalize
                        rbt[t2] = rbps.tile([K, MF], F32, tag="rb", name=f"rb{t2}")
                        nc.tensor.matmul(
                            rbt[t2], ones_row[:, :K], rt[t2],
                            start=True, stop=True,
                        )
                        ent[t2] = enp.tile([K, MF], F32R, tag="en", name=f"en{t2}")
                        nc.vector.tensor_tensor(
                            ent[t2], rbt[t2], et[t2], mybir.AluOpType.mult,
                        )
                        del rbt[t2], rt[t2], et[t2], xt[t2]
                    if 0 <= t3 < NMT:
                        # S3: out matmuls directly in the u8 domain (scale +
                        # bias folded into wvtq), pure f32->u8 copies, DMA
                        ps_a = outaps.tile([P, 2, MF], F32, tag="psa")
                        ps_b = outbps.tile([P, 2, MF], F32, tag="psb")
                        for oi in range(O_CH):
                            dst = ps_a if oi < 2 else ps_b
                            nc.tensor.matmul(
                                dst[:, oi % 2, :],
                                wvtq[:, oi * P:(oi + 1) * P],
                                ent[t3],
                                start=True, stop=True,
                            )
                        u8 = outp.tile([P, O_CH, MF], U8, tag="out")
                        nc.scalar.activation(
                            u8[:, 0:2, :], ps_a,
                            mybir.ActivationFunctionType.Identity,
                        )
                        nc.vector.tensor_copy(u8[:, 2, :], ps_b[:, 0, :])
                        nc.scalar.activation(
                            u8[:, 3, 0:416], ps_b[:, 1, 0:416],
                            mybir.ActivationFunctionType.Identity,
                        )
                        nc.vector.tensor_copy(
                            u8[:, 3, 416:MF], ps_b[:, 1, 416:MF],
                        )
                        nc.sync.dma_start(
                            out_r[:, :, t3 * MF:(t3 + 1) * MF], u8,
                        )
                        del ent[t3]

    nc.compile()
    return nc


def _get_nc():
    global _CACHED
    if _CACHED is None:
        _CACHED = _build()
    return _CACHED


def kernel(x, proxy, Wq, bq, Wk, bk, Wv, bv, Wo, bo, **run_kwargs):
    nc = _get_nc()

    crow = np.concatenate(
        [np.asarray(bk, np.float32).reshape(1, CK),
         np.asarray(bv, np.float32).reshape(1, CV),
         np.ones((1, 256), np.float32),
         np.asarray(bo, np.float32).reshape(1, C)], axis=1)
    bqbo = np.concatenate(
        [np.asarray(bq, np.float32).reshape(2, P).T,
         np.asarray(bo, np.float32).reshape(4, P).T], axis=1)
    w16 = np.concatenate(
        [np.asarray(Wk).T, np.asarray(Wv).T], axis=1
    ).astype(np.float16)
    shared = {
        "woT": np.ascontiguousarray(Wo.T).astype(np.float16),
        "wq": np.ascontiguousarray(Wq).astype(np.float16),
        "bqs16": np.ascontiguousarray(
            (np.asarray(bq, np.float32) * SCALE).reshape(2, P).T
        ).astype(np.float16),
        "crow": np.ascontiguousarray(crow),
        "bqbo": np.ascontiguousarray(bqbo),
        "ones_col": np.ones((K, 1), np.float32),
    }
    in_maps = []
    for b in range(B):
        m = dict(shared)
        m["x"] = np.ascontiguousarray(x[b]).reshape(C, N).astype(np.float16)
        pf16 = np.asarray(proxy[b, :, :, 0]).astype(np.float16)
        m["pack16"] = np.ascontiguousarray(np.concatenate([pf16, w16], axis=1))
        in_maps.append(m)

    res = bass_utils.run_bass_kernel_spmd(
        nc, in_maps, core_ids=list(range(B)), **run_kwargs
    )
    kernel.last_results = res
    out = np.empty((B, C, N), np.float32)
    for b in range(B):
        u8 = res.results[b]["out"].astype(np.float32)
        sinv = np.asarray(res.results[b]["sinv"], np.float32).reshape(C)
        out[b] = (u8 - 128.0) * sinv[:, None]
    return out.reshape(B, C, H, W)


# revision 21
# speedup vs baseline: 1.3767x; 1.0515x over previous
"""ObjectAttentionBlock2D TRN2 kernel.

Reference computation (per batch b):
    xf    = x[b].reshape(C, N)                  # C=512, N=128*128=16384
    pf    = proxy[b,:,:,0]                      # [C, K], K=64
    query = Wq @ xf + bq                        # [Ck=256, N]
    keym  = Wk @ pf + bk                        # [Ck, K]
    value = (Wv @ pf + bv).T                    # [K, Cv=256]
    sim   = softmax_k(query.T @ keym / 16)      # [N, K]
    ctx   = sim @ value                         # [N, Cv]
    out   = Wo @ ctx.T + bo                     # [C, N]

Sharding: data-parallel over batch. B=8 batches -> 8 NeuronCores, one image
per core, no collectives. Weights are replicated (host pre-transposes them so
the contraction dim is the SBUF partition dim).

Key algebraic optimization: the attention-logit and output maps are both
rank-K (K=64), and query/ctx each feed exactly one matmul, so both
projections fold into small precomputed matrices (on-device, per core):
  M     = Wq^T @ keym            [C, K]   -> simT = M^T x (4 MMs, was 10)
  sbias = (bq/16)^T @ keym       [K, 1]   -> rides in exp's bias slot
  WVT   = (Wo @ value^T)^T       [K, C]   -> out = WVT^T expPn (4 MMs, was 10)

The kernel is DMA-bound (in the TimelineSim cost model every DMA serializes
on one 360 GB/s DMA-engine pool), so both HBM streams are compressed:
  - x  is fp16 (halves the input stream; 10-bit mantissa suffices; fp8
    x was measured at 1.7e-2 end-to-end error vs the 2e-2 gate - too
    close, because logit quantization noise is amplified by softmax).
  - out is uint8 with an exact per-channel scale: out[c,:] is a convex
    combination (softmax weights) of WVT[:,c] entries plus bo[c], so
    bound_c = max_k |WVT[k,c] + bo[c]| bounds |out[c,:]| EXACTLY. The
    device computes bound_c (8 extra tiny matmuls give WVT^T in the
    partition layout of the output, then a free-axis abs-max), quantizes
    u8 = s_c*out + s_c*bo + 128 in the output-stage scale/bias slots
    (s_c = 126.5/bound_c), and ships sinv_c = bound_c/126.5 back; the
    host dequantizes (u8 - 128) * sinv. HW converts f32->u8 with
    round-to-nearest (verified empirically: offset 128.0 beats 127.5/
    128.5 2x). Output quantization adds only bounded absolute error (no
    softmax amplification): measured end-to-end rel err ~6.5e-3.

Engine-cost model (TimelineSim): every ACT/DVE instruction costs
~125-185ns fixed + ~1ns/free-elem, and a matmul costs out_free_size x
0.44ns regardless of contraction rows. At F=256 the per-instruction
fixed costs made DVE (104us) and ACT (78us) the critical path, so the
pipeline runs F=512 tiles: per 512-px tile 10 MMs, 1 exp, 1 recip, 1
en-mult, and the 4 quantize chunks are spread ACT/ACT/DVE/Pool to
balance engines. Predicted busy: DMA 74us > PE ~69 > DVE ~63 > Pool
~60 > ACT ~59 -> DMA-bound again.

Per-core pipeline over 32 macro-tiles of MF=512 pixels:
  x DMA [128, 4, 512] fp16 on gpsimd/SWDGE (1024B descriptors; >=512B
    dodges the sub-512B read-modify-write 2x penalty, and one dispatch
    per 512 px halves SWDGE descriptor-generation time on Pool)
  simT [K=64, 512] (4 fp16 MMs, contract C=512, lhsT=M)
  ACT exp(sim/16 + sbias) -> f32r SBUF
  denom = ones64^T @ expP (1 MM) -> DVE reciprocal -> K=1 broadcast MM
  expPn = expP * recip (DVE)
  out [C, 512] (4 f32r MMs, contract K=64, lhsT=WVT) -> quantize 4
    chunks (ACT/ACT/DVE/Pool) -> u8 SBUF -> one DMA on the SP HWDGE
    queue (512B descriptors)
PSUM: sim 2 bufs x 1 bank + den 1 + rb 1 + outps 1 buf x 4 banks = 8.
keym/V2/M/sbias/WVT/bounds are precomputed once per core with biases
folded in via K=1 matmul accumulation (bias outer-product with ones).

Precision: x/Wq/Wk/Wv/Wo/pf are cast to fp16 on the host; everything
downstream runs float32r (1 cycle/row at free>=256) with fp32 PSUM
accumulation. Output u8 + per-channel scale as above.
"""

import numpy as np

import concourse.bacc as bacc
import concourse.mybir as mybir
import concourse.tile as tile
from concourse import bass_utils

F32 = mybir.dt.float32
F32R = mybir.dt.float32r
F16 = mybir.dt.float16
U8 = mybir.dt.uint8

B, C, H, W = 8, 512, 128, 128
N = H * W                    # 16384 pixels per image
CK, CV, K = 256, 256, 64
P = 128                      # SBUF partitions
MF = 512                     # pixel tile width (compute + DMA)
NMT = N // MF                # 32 tiles
CI_CH = C // P               # 4 contraction chunks over C
Q_CH = CK // P               # 2 chunks over Ck
V_CH = CV // P               # 2 chunks over Cv
O_CH = C // P                # 4 chunks over output C
SCALE = CK ** -0.5           # 1/16
QCAP = 126.5                 # |s*out| <= 126.5 so u8 = s*out+128 in [1.5, 254.5]

_CACHED = None


def _build():
    nc = bacc.Bacc("TRN2", target_bir_lowering=False, debug=False)

    X = nc.dram_tensor("x", [C, N], F16, kind="ExternalInput").ap()
    # pack16[c, :] = [pf(64) | wkT(256) | wvT(256)] in fp16
    PACK16 = nc.dram_tensor("pack16", [C, 576], F16, kind="ExternalInput").ap()
    WQ = nc.dram_tensor("wq", [CK, C], F16, kind="ExternalInput").ap()
    # crow = [bk(256) | bv(256) | ones(256) | bo(512)] as one row
    CROW = nc.dram_tensor("crow", [1, 1280], F32, kind="ExternalInput").ap()
    ONESC = nc.dram_tensor("ones_col", [K, 1], F32, kind="ExternalInput").ap()
    # bqbo[p, :] = [bq 2 chunks | bo 4 chunks] per-partition layout
    BQBO = nc.dram_tensor("bqbo", [P, 6], F32, kind="ExternalInput").ap()
    BQS16 = nc.dram_tensor("bqs16", [P, 2], F16, kind="ExternalInput").ap()
    WOT = nc.dram_tensor("woT", [CV, C], F16, kind="ExternalInput").ap()
    OUT = nc.dram_tensor("out", [C, N], U8, kind="ExternalOutput").ap()
    SINV = nc.dram_tensor("sinv", [1, C], F32, kind="ExternalOutput").ap()

    x_r = X.rearrange("(co p) n -> p co n", p=P)       # [128, 4, N]
    out_r = OUT.rearrange("(oo p) n -> p oo n", p=P)   # [128, 4, N]

    with tile.TileContext(nc) as tc:
        with tc.tile_pool(name="const", bufs=1) as cp:
            pack = cp.tile([P, CI_CH, 576], F16)
            nc.sync.dma_start(pack, PACK16.rearrange("(co p) q -> p co q", p=P))
            pf = pack[:, :, 0:K]
            wk = pack[:, :, K:K + CK]
            wv = pack[:, :, K + CK:K + CK + CV]
            wq = cp.tile([P, Q_CH, C], F16)
            nc.sync.dma_start(wq, WQ.rearrange("(qo p) c -> p qo c", p=P))
            crow = cp.tile([1, 1280], F32R)
            nc.sync.dma_start(crow, CROW.bitcast(F32R))
            bk_row = crow[:, 0:CK]
            bv_row = crow[:, CK:CK + CV]
            ones_row = crow[:, 512:768]
            bo_row = crow[:, 768:1280]
            ones_col = cp.tile([K, 1], F32R)
            nc.sync.dma_start(ones_col, ONESC.bitcast(F32R))
            bqbo = cp.tile([P, 6], F32)
            nc.scalar.dma_start(bqbo, BQBO)
            bqs16 = cp.tile([P, 2], F16)
            nc.scalar.dma_start(bqs16, BQS16)
            bqs = bqs16
            bo = bqbo[:, 2:6]
            wo = cp.tile([P, V_CH, C], F16)
            nc.scalar.dma_start(wo, WOT.rearrange("(vo p) o -> p vo o", p=P))

            keym = cp.tile([P, Q_CH, K], F16)    # [q-part, q-chunk, k]
            msim = cp.tile([P, CI_CH, K], F16)   # M[c,k] = sum_q Wq[q,c]*keym[q,k]
            sbias = cp.tile([K, 1], F32)         # sum_q (bq[q]/16)*keym[q,k]
            # u8 quantization: out[c,:] is a convex combination (softmax
            # weights sum to 1) of WVT[:,c]+bo[c], so with
            #   wvtq[k,c] = s_c*(WVT[k,c]+bo[c]) + 128,  s_c = 126.5/bound_c
            # the out matmul directly yields u8-domain values in [1.5,254.5]
            # (the +128 rides on sum_k en = 1) and the output stage is a pure
            # f32->u8 copy with no per-channel scalars.
            wvtb = cp.tile([K, C], F32R)         # WVT[k,c] + bo[c]
            wvtq = cp.tile([K, C], F32R)         # s_c*(WVT+bo) + 128
            bound_row = cp.tile([1, C], F32R)    # max_k |WVT[k,c]+bo[c]|
            r_row = cp.tile([1, C], F32R)
            s_row = cp.tile([1, C], F32R)        # 126.5 / bound
            sinv_row = cp.tile([1, C], F32R)     # bound / 126.5

            # ---- one-time: keym = Wk @ pf + bk, value[k,v] = (Wv @ pf + bv)[v,k]
            with tc.tile_pool(name="setup_ps", bufs=1, space="PSUM") as sps:
                kps = sps.tile([P, Q_CH, K], F32)
                for qi in range(Q_CH):
                    for ci in range(CI_CH):
                        nc.tensor.matmul(
                            kps[:, qi, :],
                            wk[:, ci, qi * P:(qi + 1) * P],
                            pf[:, ci, :],
                            start=(ci == 0), stop=False,
                        )
                    # += bk[q] * ones[k]
                    nc.tensor.matmul(
                        kps[:, qi, :],
                        bk_row[:, qi * P:(qi + 1) * P],
                        ones_row[:, :K],
                        start=False, stop=True,
                    )
                nc.vector.tensor_copy(keym, kps)

                v2ps = sps.tile([P, V_CH, K], F32)
                for vi in range(V_CH):
                    for ci in range(CI_CH):
                        nc.tensor.matmul(
                            v2ps[:, vi, :],
                            wv[:, ci, vi * P:(vi + 1) * P],
                            pf[:, ci, :],
                            start=(ci == 0), stop=False,
                        )
                    nc.tensor.matmul(
                        v2ps[:, vi, :],
                        bv_row[:, vi * P:(vi + 1) * P],
                        ones_row[:, :K],
                        start=False, stop=True,
                    )
                v2sb = cp.tile([P, V_CH, K], F16)
                nc.vector.tensor_copy(v2sb, v2ps)
                wvtps = sps.tile([K, C], F32)
                for vi in range(V_CH):
                    nc.tensor.matmul(
                        wvtps, v2sb[:, vi, :], wo[:, vi, :],
                        start=(vi == 0), stop=False,
                    )
                # += bo broadcast along k (rank-1 accumulate) -> WVT + bo
                nc.tensor.matmul(
                    wvtps, ones_row[:, :K], bo_row, start=False, stop=True,
                )
                with nc.allow_low_precision(reason="f32r is 4-byte fp32"):
                    nc.vector.tensor_copy(wvtb, wvtps)
                    # exact |out| bound per channel: abs on DVE (gpsimd's
                    # apply_absolute_value is ignored on HW), then
                    # cross-partition max on gpsimd
                    wvta = cp.tile([K, C], F32R)
                    nc.vector.tensor_scalar(
                        wvta, wvtb, -1.0, None, op0=mybir.AluOpType.mult,
                    )
                    nc.vector.tensor_tensor(wvta, wvtb, wvta, mybir.AluOpType.max)
                    nc.gpsimd.tensor_reduce(
                        bound_row, wvta, axis=mybir.AxisListType.C,
                        op=mybir.AluOpType.max,
                    )
                    nc.vector.tensor_scalar(
                        bound_row, bound_row, 1e-3, None, op0=mybir.AluOpType.max,
                    )
                    nc.vector.reciprocal(r_row, bound_row)
                    nc.vector.tensor_scalar(
                        s_row, r_row, QCAP, None, op0=mybir.AluOpType.mult,
                    )
                    nc.vector.tensor_scalar(
                        sinv_row, bound_row, 1.0 / QCAP, None,
                        op0=mybir.AluOpType.mult,
                    )
                    nc.scalar.dma_start(SINV, sinv_row.bitcast(F32))
                    # wvtq = s*(WVT+bo) + 128, via rank-1 broadcast of s
                    s_bc = sps.tile([K, C], F32)
                    nc.tensor.matmul(s_bc, ones_row[:, :K], s_row, start=True, stop=True)
                    nc.vector.tensor_tensor(wvtq, wvtb, s_bc, mybir.AluOpType.mult)
                    nc.vector.tensor_scalar(
                        wvtq, wvtq, 128.0, None, op0=mybir.AluOpType.add,
                    )

                # M: fold the Q projection into the sim matmul (Q only feeds sim)
                mps = sps.tile([P, CI_CH, K], F32)
                for ci in range(CI_CH):
                    for qi in range(Q_CH):
                        nc.tensor.matmul(
                            mps[:, ci, :],
                            wq[:, qi, ci * P:(ci + 1) * P],
                            keym[:, qi, :],
                            start=(qi == 0), stop=(qi == Q_CH - 1),
                        )
                nc.vector.tensor_copy(msim, mps)
                # sbias[k,1]: lhsT=keym chunks, rhs=bq/16 column
                sbps = sps.tile([K, 1], F32)
                for qi in range(Q_CH):
                    nc.tensor.matmul(
                        sbps, keym[:, qi, :], bqs[:, qi:qi + 1],
                        start=(qi == 0), stop=(qi == Q_CH - 1),
                    )
                nc.vector.tensor_copy(sbias, sbps)

            # ---- steady-state: 4-stage software pipeline over 512-px tiles
            # Iteration i runs  S0: sim+exp(i) | S1: den+recip(i-1) |
            # S2: rb+en(i-2) | S3: out MMs + u8 copies + DMA(i-3).
            # The softmax chain has 3 cross-engine round trips; skewing the
            # stages gives every dependency a full iteration of slack so each
            # in-order engine queue never waits (the naive fused loop ran all
            # engines at ~50% on exactly those round trips).
            # Copy split tuned to balance ACT (exp + 1038 + 532 = 2182ns/it)
            # vs DVE (recip + en + 658 + 225 = 2199ns/it), both under the
            # 2287ns/it DMA floor.
            PRE = 3   # x-DMA prefetch distance (iterations)
            with (
                tc.tile_pool(name="xin", bufs=6) as xp,
                tc.tile_pool(name="esb", bufs=4) as ep,
                tc.tile_pool(name="densb", bufs=2) as dnp,
                tc.tile_pool(name="rsb", bufs=3) as rp,
                tc.tile_pool(name="ensb", bufs=3) as enp,
                tc.tile_pool(name="outsb", bufs=4) as outp,
                tc.tile_pool(name="sdps", bufs=1, space="PSUM") as sdps,
                tc.tile_pool(name="rbps", bufs=1, space="PSUM") as rbps,
                tc.tile_pool(name="outaps", bufs=1, space="PSUM") as outaps,
                tc.tile_pool(name="outbps", bufs=2, space="PSUM") as outbps,
            ):
                xt = {}
                et = {}
                rt = {}
                dent = {}
                rbt = {}
                ent = {}

                def fetch_x(t):
                    # HWDGE on the SP queue, which carries only DMA
                    # dispatches that never wait (x deps are always ready,
                    # and out-DMAs are dispatched a full iteration after
                    # their copies complete) — SWDGE's 1024-descriptor ring
                    # (= 2 of these DMAs) serialized gen->transfer at
                    # 3.3us/tile, and compute-queue dispatch inherits the
                    # preceding op's semaphore waits.
                    if t < NMT:
                        xt[t] = xp.tile([P, CI_CH, MF], F16, tag="x", name=f"x{t}")
                        nc.sync.dma_start(xt[t], x_r[:, :, t * MF:(t + 1) * MF])

                for t in range(PRE):
                    fetch_x(t)

                u8t = {}
                for i in range(NMT + 4):
                    fetch_x(i + PRE)
                    t0, t1, t2, t3, t4 = i, i - 1, i - 2, i - 3, i - 4
                    if t0 < NMT:
                        # S0: simT[k,n] = M^T-contract-c @ x, then exp
                        sim = sdps.tile([K, MF], F32, tag="sd")
                        for ci in range(CI_CH):
                            nc.tensor.matmul(
                                sim, msim[:, ci, :], xt[t0][:, ci, :],
                                start=(ci == 0), stop=(ci == CI_CH - 1),
                            )
                        et[t0] = ep.tile([K, MF], F32R, tag="e", name=f"e{t0}")
                        nc.scalar.activation(
                            et[t0], sim, mybir.ActivationFunctionType.Exp,
                            scale=SCALE, bias=sbias,
                        )
                    if 0 <= t1 < NMT:
                        # S1: denom on the otherwise-idle Pool engine (cross-
                        # partition sum, SBUF->SBUF — keeps PSUM banks and the
                        # PE free), then reciprocal on DVE
                        dent[t1] = dnp.tile([1, MF], F32R, tag="den", name=f"den{t1}")
                        with nc.allow_low_precision(reason="f32r is 4-byte fp32"):
                            nc.gpsimd.tensor_reduce(
                                dent[t1], et[t1], axis=mybir.AxisListType.C,
                                op=mybir.AluOpType.add,
                            )
                        rt[t1] = rp.tile([1, MF], F32R, tag="r", name=f"r{t1}")
                        with nc.allow_low_precision(reason="f32r is 4-byte fp32"):
                            nc.vector.reciprocal(rt[t1], dent[t1])
                        del dent[t1]
                    if 0 <= t2 < NMT:
                        # S2: broadcast recip over k, normalize
                        rbt[t2] = rbps.tile([K, MF], F32, tag="rb", name=f"rb{t2}")
                        nc.tensor.matmul(
                            rbt[t2], ones_row[:, :K], rt[t2],
                            start=True, stop=True,
                        )
                        ent[t2] = enp.tile([K, MF], F32R, tag="en", name=f"en{t2}")
                        nc.vector.tensor_tensor(
                            ent[t2], rbt[t2], et[t2], mybir.AluOpType.mult,
                        )
                        del rbt[t2], rt[t2], et[t2], xt[t2]
                    if 0 <= t3 < NMT:
                        # S3: out matmuls directly in the u8 domain (scale +
                        # bias folded into wvtq), pure f32->u8 copies, DMA
                        ps_a = outaps.tile([P, 2, MF], F32, tag="psa")
                        ps_b = outbps.tile([P, 2, MF], F32, tag="psb")
                        for oi in range(O_CH):
                            dst = ps_a if oi < 2 else ps_b
                            nc.tensor.matmul(
                                dst[:, oi % 2, :],
                                wvtq[:, oi * P:(oi + 1) * P],
                                ent[t3],
                                start=True, stop=True,
                            )
                        u8 = outp.tile([P, O_CH, MF], U8, tag="out", name=f"u8{t3}")
                        nc.scalar.activation(
                            u8[:, 0:2, :], ps_a,
                            mybir.ActivationFunctionType.Identity,
                        )
                        nc.vector.tensor_copy(u8[:, 2, :], ps_b[:, 0, :])
                        nc.scalar.activation(
                            u8[:, 3, 0:416], ps_b[:, 1, 0:416],
                            mybir.ActivationFunctionType.Identity,
                        )
                        nc.vector.tensor_copy(
                            u8[:, 3, 416:MF], ps_b[:, 1, 416:MF],
                        )
                        u8t[t3] = u8
                        del ent[t3]
                    if 0 <= t4 < NMT:
                        # S4: the out-DMA dispatches an iteration after its
                        # copies completed, so the SP queue never stalls
                        nc.sync.dma_start(
                            out_r[:, :, t4 * MF:(t4 + 1) * MF], u8t.pop(t4),
                        )

    nc.compile()
    return nc


def _get_nc():
    global _CACHED
    if _CACHED is None:
        _CACHED = _build()
    return _CACHED


def kernel(x, proxy, Wq, bq, Wk, bk, Wv, bv, Wo, bo, **run_kwargs):
    nc = _get_nc()

    crow = np.concatenate(
        [np.asarray(bk, np.float32).reshape(1, CK),
         np.asarray(bv, np.float32).reshape(1, CV),
         np.ones((1, 256), np.float32),
         np.asarray(bo, np.float32).reshape(1, C)], axis=1)
    bqbo = np.concatenate(
        [np.asarray(bq, np.float32).reshape(2, P).T,
         np.asarray(bo, np.float32).reshape(4, P).T], axis=1)
    w16 = np.concatenate(
        [np.asarray(Wk).T, np.asarray(Wv).T], axis=1
    ).astype(np.float16)
    shared = {
        "woT": np.ascontiguousarray(Wo.T).astype(np.float16),
        "wq": np.ascontiguousarray(Wq).astype(np.float16),
        "bqs16": np.ascontiguousarray(
            (np.asarray(bq, np.float32) * SCALE).reshape(2, P).T
        ).astype(np.float16),
        "crow": np.ascontiguousarray(crow),
        "bqbo": np.ascontiguousarray(bqbo),
        "ones_col": np.ones((K, 1), np.float32),
    }
    in_maps = []
    for b in range(B):
        m = dict(shared)
        m["x"] = np.ascontiguousarray(x[b]).reshape(C, N).astype(np.float16)
        pf16 = np.asarray(proxy[b, :, :, 0]).astype(np.float16)
        m["pack16"] = np.ascontiguousarray(np.concatenate([pf16, w16], axis=1))
        in_maps.append(m)

    res = bass_utils.run_bass_kernel_spmd(
        nc, in_maps, core_ids=list(range(B)), **run_kwargs
    )
    kernel.last_results = res
    out = np.empty((B, C, N), np.float32)
    for b in range(B):
        u8 = res.results[b]["out"].astype(np.float32)
        sinv = np.asarray(res.results[b]["sinv"], np.float32).reshape(C)
        out[b] = (u8 - 128.0) * sinv[:, None]
    return out.reshape(B, C, H, W)


# revision 22
# speedup vs baseline: 1.4679x; 1.0663x over previous
"""ObjectAttentionBlock2D TRN2 kernel.

Reference computation (per batch b):
    xf    = x[b].reshape(C, N)                  # C=512, N=128*128=16384
    pf    = proxy[b,:,:,0]                      # [C, K], K=64
    query = Wq @ xf + bq                        # [Ck=256, N]
    keym  = Wk @ pf + bk                        # [Ck, K]
    value = (Wv @ pf + bv).T                    # [K, Cv=256]
    sim   = softmax_k(query.T @ keym / 16)      # [N, K]
    ctx   = sim @ value                         # [N, Cv]
    out   = Wo @ ctx.T + bo                     # [C, N]

Sharding: data-parallel over batch. B=8 batches -> 8 NeuronCores, one image
per core, no collectives. Weights are replicated (host pre-transposes them so
the contraction dim is the SBUF partition dim).

Key algebraic optimization: the attention-logit and output maps are both
rank-K (K=64), and query/ctx each feed exactly one matmul, so both
projections fold into small precomputed matrices (on-device, per core):
  M     = Wq^T @ keym            [C, K]   -> simT = M^T x (4 MMs, was 10)
  sbias = (bq/16)^T @ keym       [K, 1]   -> rides in exp's bias slot
  WVT   = (Wo @ value^T)^T       [K, C]   -> out = WVT^T expPn (4 MMs, was 10)

The kernel is DMA-bound (in the TimelineSim cost model every DMA serializes
on one 360 GB/s DMA-engine pool), so both HBM streams are compressed:
  - x  is fp16 (halves the input stream; 10-bit mantissa suffices; fp8
    x was measured at 1.7e-2 end-to-end error vs the 2e-2 gate - too
    close, because logit quantization noise is amplified by softmax).
  - out is uint8 with an exact per-channel scale: out[c,:] is a convex
    combination (softmax weights) of WVT[:,c] entries plus bo[c], so
    bound_c = max_k |WVT[k,c] + bo[c]| bounds |out[c,:]| EXACTLY. The
    device computes bound_c (8 extra tiny matmuls give WVT^T in the
    partition layout of the output, then a free-axis abs-max), quantizes
    u8 = s_c*out + s_c*bo + 128 in the output-stage scale/bias slots
    (s_c = 126.5/bound_c), and ships sinv_c = bound_c/126.5 back; the
    host dequantizes (u8 - 128) * sinv. HW converts f32->u8 with
    round-to-nearest (verified empirically: offset 128.0 beats 127.5/
    128.5 2x). Output quantization adds only bounded absolute error (no
    softmax amplification): measured end-to-end rel err ~6.5e-3.

Engine-cost model (TimelineSim): every ACT/DVE instruction costs
~125-185ns fixed + ~1ns/free-elem, and a matmul costs out_free_size x
0.44ns regardless of contraction rows. At F=256 the per-instruction
fixed costs made DVE (104us) and ACT (78us) the critical path, so the
pipeline runs F=512 tiles: per 512-px tile 10 MMs, 1 exp, 1 recip, 1
en-mult, and the 4 quantize chunks are spread ACT/ACT/DVE/Pool to
balance engines. Predicted busy: DMA 74us > PE ~69 > DVE ~63 > Pool
~60 > ACT ~59 -> DMA-bound again.

Per-core pipeline over 32 macro-tiles of MF=512 pixels:
  x DMA [128, 4, 512] fp16 on gpsimd/SWDGE (1024B descriptors; >=512B
    dodges the sub-512B read-modify-write 2x penalty, and one dispatch
    per 512 px halves SWDGE descriptor-generation time on Pool)
  simT [K=64, 512] (4 fp16 MMs, contract C=512, lhsT=M)
  ACT exp(sim/16 + sbias) -> f32r SBUF
  denom = ones64^T @ expP (1 MM) -> DVE reciprocal -> K=1 broadcast MM
  expPn = expP * recip (DVE)
  out [C, 512] (4 f32r MMs, contract K=64, lhsT=WVT) -> quantize 4
    chunks (ACT/ACT/DVE/Pool) -> u8 SBUF -> one DMA on the SP HWDGE
    queue (512B descriptors)
PSUM: sim 2 bufs x 1 bank + den 1 + rb 1 + outps 1 buf x 4 banks = 8.
keym/V2/M/sbias/WVT/bounds are precomputed once per core with biases
folded in via K=1 matmul accumulation (bias outer-product with ones).

Precision: x/Wq/Wk/Wv/Wo/pf are cast to fp16 on the host; everything
downstream runs float32r (1 cycle/row at free>=256) with fp32 PSUM
accumulation. Output u8 + per-channel scale as above.
"""

import numpy as np

import concourse.bacc as bacc
import concourse.mybir as mybir
import concourse.tile as tile
from concourse import bass_utils

F32 = mybir.dt.float32
F32R = mybir.dt.float32r
F16 = mybir.dt.float16
U8 = mybir.dt.uint8

B, C, H, W = 8, 512, 128, 128
N = H * W                    # 16384 pixels per image
CK, CV, K = 256, 256, 64
P = 128                      # SBUF partitions
MF = 512                     # pixel tile width (compute + DMA)
NMT = N // MF                # 32 tiles
CI_CH = C // P               # 4 contraction chunks over C
Q_CH = CK // P               # 2 chunks over Ck
V_CH = CV // P               # 2 chunks over Cv
O_CH = C // P                # 4 chunks over output C
SCALE = CK ** -0.5           # 1/16
QCAP = 126.5                 # |s*out| <= 126.5 so u8 = s*out+128 in [1.5, 254.5]

_CACHED = None


def _build():
    nc = bacc.Bacc("TRN2", target_bir_lowering=False, debug=False)

    X = nc.dram_tensor("x", [C, N], F16, kind="ExternalInput").ap()
    # pack16[c, :] = [pf(64) | wkT(256) | wvT(256)] in fp16
    PACK16 = nc.dram_tensor("pack16", [C, 576], F16, kind="ExternalInput").ap()
    WQ = nc.dram_tensor("wq", [CK, C], F16, kind="ExternalInput").ap()
    # crow = [bk(256) | bv(256) | ones(256) | bo(512)] as one row
    CROW = nc.dram_tensor("crow", [1, 1280], F32, kind="ExternalInput").ap()
    ONESC = nc.dram_tensor("ones_col", [K, 1], F32, kind="ExternalInput").ap()
    # bqbo[p, :] = [bq 2 chunks | bo 4 chunks] per-partition layout
    BQBO = nc.dram_tensor("bqbo", [P, 6], F32, kind="ExternalInput").ap()
    BQS16 = nc.dram_tensor("bqs16", [P, 2], F16, kind="ExternalInput").ap()
    WOT = nc.dram_tensor("woT", [CV, C], F16, kind="ExternalInput").ap()
    OUT = nc.dram_tensor("out", [C, N], U8, kind="ExternalOutput").ap()
    SINV = nc.dram_tensor("sinv", [1, C], F32, kind="ExternalOutput").ap()

    x_r = X.rearrange("(co p) n -> p co n", p=P)       # [128, 4, N]
    out_r = OUT.rearrange("(oo p) n -> p oo n", p=P)   # [128, 4, N]

    with tile.TileContext(nc) as tc:
        with tc.tile_pool(name="const", bufs=1) as cp:
            pack = cp.tile([P, CI_CH, 576], F16)
            nc.sync.dma_start(pack, PACK16.rearrange("(co p) q -> p co q", p=P))
            pf = pack[:, :, 0:K]
            wk = pack[:, :, K:K + CK]
            wv = pack[:, :, K + CK:K + CK + CV]
            wq = cp.tile([P, Q_CH, C], F16)
            nc.sync.dma_start(wq, WQ.rearrange("(qo p) c -> p qo c", p=P))
            crow = cp.tile([1, 1280], F32R)
            nc.sync.dma_start(crow, CROW.bitcast(F32R))
            bk_row = crow[:, 0:CK]
            bv_row = crow[:, CK:CK + CV]
            ones_row = crow[:, 512:768]
            bo_row = crow[:, 768:1280]
            ones_col = cp.tile([K, 1], F32R)
            nc.sync.dma_start(ones_col, ONESC.bitcast(F32R))
            bqbo = cp.tile([P, 6], F32)
            nc.scalar.dma_start(bqbo, BQBO)
            bqs16 = cp.tile([P, 2], F16)
            nc.scalar.dma_start(bqs16, BQS16)
            bqs = bqs16
            bo = bqbo[:, 2:6]
            wo = cp.tile([P, V_CH, C], F16)
            nc.scalar.dma_start(wo, WOT.rearrange("(vo p) o -> p vo o", p=P))

            keym = cp.tile([P, Q_CH, K], F16)    # [q-part, q-chunk, k]
            msim = cp.tile([P, CI_CH, K], F16)   # M[c,k] = sum_q Wq[q,c]*keym[q,k]
            sbias = cp.tile([K, 1], F32)         # sum_q (bq[q]/16)*keym[q,k]
            # u8 quantization: out[c,:] is a convex combination (softmax
            # weights sum to 1) of WVT[:,c]+bo[c], so with
            #   wvtq[k,c] = s_c*(WVT[k,c]+bo[c]) + 128,  s_c = 126.5/bound_c
            # the out matmul directly yields u8-domain values in [1.5,254.5]
            # (the +128 rides on sum_k en = 1) and the output stage is a pure
            # f32->u8 copy with no per-channel scalars.
            wvtb = cp.tile([K, C], F32R)         # WVT[k,c] + bo[c]
            wvtq = cp.tile([K, C], F32R)         # s_c*(WVT+bo) + 128
            bound_row = cp.tile([1, C], F32R)    # max_k |WVT[k,c]+bo[c]|
            r_row = cp.tile([1, C], F32R)
            s_row = cp.tile([1, C], F32R)        # 126.5 / bound
            sinv_row = cp.tile([1, C], F32R)     # bound / 126.5

            # ---- one-time: keym = Wk @ pf + bk, value[k,v] = (Wv @ pf + bv)[v,k]
            with tc.tile_pool(name="setup_ps", bufs=1, space="PSUM") as sps:
                kps = sps.tile([P, Q_CH, K], F32)
                for qi in range(Q_CH):
                    for ci in range(CI_CH):
                        nc.tensor.matmul(
                            kps[:, qi, :],
                            wk[:, ci, qi * P:(qi + 1) * P],
                            pf[:, ci, :],
                            start=(ci == 0), stop=False,
                        )
                    # += bk[q] * ones[k]
                    nc.tensor.matmul(
                        kps[:, qi, :],
                        bk_row[:, qi * P:(qi + 1) * P],
                        ones_row[:, :K],
                        start=False, stop=True,
                    )
                nc.vector.tensor_copy(keym, kps)

                v2ps = sps.tile([P, V_CH, K], F32)
                for vi in range(V_CH):
                    for ci in range(CI_CH):
                        nc.tensor.matmul(
                            v2ps[:, vi, :],
                            wv[:, ci, vi * P:(vi + 1) * P],
                            pf[:, ci, :],
                            start=(ci == 0), stop=False,
                        )
                    nc.tensor.matmul(
                        v2ps[:, vi, :],
                        bv_row[:, vi * P:(vi + 1) * P],
                        ones_row[:, :K],
                        start=False, stop=True,
                    )
                v2sb = cp.tile([P, V_CH, K], F16)
                nc.vector.tensor_copy(v2sb, v2ps)
                wvtps = sps.tile([K, C], F32)
                for vi in range(V_CH):
                    nc.tensor.matmul(
                        wvtps, v2sb[:, vi, :], wo[:, vi, :],
                        start=(vi == 0), stop=False,
                    )
                # += bo broadcast along k (rank-1 accumulate) -> WVT + bo
                nc.tensor.matmul(
                    wvtps, ones_row[:, :K], bo_row, start=False, stop=True,
                )
                with nc.allow_low_precision(reason="f32r is 4-byte fp32"):
                    nc.vector.tensor_copy(wvtb, wvtps)
                    # exact |out| bound per channel: abs on DVE (gpsimd's
                    # apply_absolute_value is ignored on HW), then
                    # cross-partition max on gpsimd
                    wvta = cp.tile([K, C], F32R)
                    nc.vector.tensor_scalar(
                        wvta, wvtb, -1.0, None, op0=mybir.AluOpType.mult,
                    )
                    nc.vector.tensor_tensor(wvta, wvtb, wvta, mybir.AluOpType.max)
                    nc.gpsimd.tensor_reduce(
                        bound_row, wvta, axis=mybir.AxisListType.C,
                        op=mybir.AluOpType.max,
                    )
                    nc.vector.tensor_scalar(
                        bound_row, bound_row, 1e-3, None, op0=mybir.AluOpType.max,
                    )
                    nc.vector.reciprocal(r_row, bound_row)
                    nc.vector.tensor_scalar(
                        s_row, r_row, QCAP, None, op0=mybir.AluOpType.mult,
                    )
                    nc.vector.tensor_scalar(
                        sinv_row, bound_row, 1.0 / QCAP, None,
                        op0=mybir.AluOpType.mult,
                    )
                    nc.scalar.dma_start(SINV, sinv_row.bitcast(F32))
                    # wvtq = s*(WVT+bo) + 128, via rank-1 broadcast of s
                    s_bc = sps.tile([K, C], F32)
                    nc.tensor.matmul(s_bc, ones_row[:, :K], s_row, start=True, stop=True)
                    nc.vector.tensor_tensor(wvtq, wvtb, s_bc, mybir.AluOpType.mult)
                    nc.vector.tensor_scalar(
                        wvtq, wvtq, 128.0, None, op0=mybir.AluOpType.add,
                    )

                # M: fold the Q projection into the sim matmul (Q only feeds sim)
                mps = sps.tile([P, CI_CH, K], F32)
                for ci in range(CI_CH):
                    for qi in range(Q_CH):
                        nc.tensor.matmul(
                            mps[:, ci, :],
                            wq[:, qi, ci * P:(ci + 1) * P],
                            keym[:, qi, :],
                            start=(qi == 0), stop=(qi == Q_CH - 1),
                        )
                nc.vector.tensor_copy(msim, mps)
                # sbias[k,1]: lhsT=keym chunks, rhs=bq/16 column
                sbps = sps.tile([K, 1], F32)
                for qi in range(Q_CH):
                    nc.tensor.matmul(
                        sbps, keym[:, qi, :], bqs[:, qi:qi + 1],
                        start=(qi == 0), stop=(qi == Q_CH - 1),
                    )
                nc.vector.tensor_copy(sbias, sbps)

            # ---- steady-state: 4-stage software pipeline over 512-px tiles
            # Iteration i runs  S0: sim+exp(i) | S1: den+recip(i-1) |
            # S2: rb+en(i-2) | S3: out MMs + u8 copies + DMA(i-3).
            # The softmax chain has 3 cross-engine round trips; skewing the
            # stages gives every dependency a full iteration of slack so each
            # in-order engine queue never waits (the naive fused loop ran all
            # engines at ~50% on exactly those round trips).
            # Copy split tuned to balance ACT (exp + 1038 + 532 = 2182ns/it)
            # vs DVE (recip + en + 658 + 225 = 2199ns/it), both under the
            # 2287ns/it DMA floor.
            PRE = 3   # x-DMA prefetch distance (iterations)
            with (
                tc.tile_pool(name="xin", bufs=6) as xp,
                tc.tile_pool(name="esb", bufs=4) as ep,
                tc.tile_pool(name="densb", bufs=2) as dnp,
                tc.tile_pool(name="rsb", bufs=3) as rp,
                tc.tile_pool(name="ensb", bufs=3) as enp,
                tc.tile_pool(name="outsb", bufs=4) as outp,
                tc.tile_pool(name="sdps", bufs=1, space="PSUM") as sdps,
                tc.tile_pool(name="rbps", bufs=1, space="PSUM") as rbps,
                tc.tile_pool(name="outaps", bufs=1, space="PSUM") as outaps,
                tc.tile_pool(name="outb1ps", bufs=2, space="PSUM") as outb1ps,
                tc.tile_pool(name="outb2ps", bufs=2, space="PSUM") as outb2ps,
            ):
                xt = {}
                et = {}
                rt = {}
                dent = {}
                rbt = {}
                ent = {}

                def fetch_x(t):
                    # HWDGE on the SP queue, which carries only DMA
                    # dispatches that never wait (x deps are always ready,
                    # and out-DMAs are dispatched a full iteration after
                    # their copies complete) — SWDGE's 1024-descriptor ring
                    # (= 2 of these DMAs) serialized gen->transfer at
                    # 3.3us/tile, and compute-queue dispatch inherits the
                    # preceding op's semaphore waits.
                    if t < NMT:
                        xt[t] = xp.tile([P, CI_CH, MF], F16, tag="x", name=f"x{t}")
                        nc.sync.dma_start(xt[t], x_r[:, :, t * MF:(t + 1) * MF])

                for t in range(PRE):
                    fetch_x(t)

                u8t = {}
                for i in range(NMT + 4):
                    fetch_x(i + PRE)
                    t0, t1, t2, t3, t4 = i, i - 1, i - 2, i - 3, i - 4
                    if t0 < NMT:
                        # S0: simT[k,n] = M^T-contract-c @ x, then exp
                        sim = sdps.tile([K, MF], F32, tag="sd")
                        for ci in range(CI_CH):
                            nc.tensor.matmul(
                                sim, msim[:, ci, :], xt[t0][:, ci, :],
                                start=(ci == 0), stop=(ci == CI_CH - 1),
                            )
                        et[t0] = ep.tile([K, MF], F32R, tag="e", name=f"e{t0}")
                        nc.scalar.activation(
                            et[t0], sim, mybir.ActivationFunctionType.Exp,
                            scale=SCALE, bias=sbias,
                        )
                    if 0 <= t1 < NMT:
                        # S1: denom on the otherwise-idle Pool engine (cross-
                        # partition sum, SBUF->SBUF — keeps PSUM banks and the
                        # PE free), then reciprocal on DVE
                        dent[t1] = dnp.tile([1, MF], F32R, tag="den", name=f"den{t1}")
                        with nc.allow_low_precision(reason="f32r is 4-byte fp32"):
                            nc.gpsimd.tensor_reduce(
                                dent[t1], et[t1], axis=mybir.AxisListType.C,
                                op=mybir.AluOpType.add,
                            )
                        rt[t1] = rp.tile([1, MF], F32R, tag="r", name=f"r{t1}")
                        with nc.allow_low_precision(reason="f32r is 4-byte fp32"):
                            nc.vector.reciprocal(rt[t1], dent[t1])
                        del dent[t1]
                    if 0 <= t2 < NMT:
                        # S2: broadcast recip over k, normalize
                        rbt[t2] = rbps.tile([K, MF], F32, tag="rb", name=f"rb{t2}")
                        nc.tensor.matmul(
                            rbt[t2], ones_row[:, :K], rt[t2],
                            start=True, stop=True,
                        )
                        ent[t2] = enp.tile([K, MF], F32R, tag="en", name=f"en{t2}")
                        nc.vector.tensor_tensor(
                            ent[t2], rbt[t2], et[t2], mybir.AluOpType.mult,
                        )
                        del rbt[t2], rt[t2], et[t2], xt[t2]
                    if 0 <= t3 < NMT:
                        # S3: out matmuls directly in the u8 domain (scale +
                        # bias folded into wvtq), pure f32->u8 copies, DMA
                        # one copy op per PSUM tile, single consumer each:
                        # pa (2 chunks) -> ACT, pb1 -> DVE, pb2 -> ACT.
                        # ACT 612+1038+612 = 2262ns/it, DVE 658*3 = 1974ns/it
                        ps_a = outaps.tile([P, 2, MF], F32, tag="psa")
                        ps_b1 = outb1ps.tile([P, 1, MF], F32, tag="psb1")
                        ps_b2 = outb2ps.tile([P, 1, MF], F32, tag="psb2")
                        chunks = [ps_a[:, 0, :], ps_a[:, 1, :],
                                  ps_b1[:, 0, :], ps_b2[:, 0, :]]
                        for oi in range(O_CH):
                            nc.tensor.matmul(
                                chunks[oi],
                                wvtq[:, oi * P:(oi + 1) * P],
                                ent[t3],
                                start=True, stop=True,
                            )
                        u8 = outp.tile([P, O_CH, MF], U8, tag="out", name=f"u8{t3}")
                        nc.scalar.activation(
                            u8[:, 0:2, :], ps_a,
                            mybir.ActivationFunctionType.Identity,
                        )
                        nc.vector.tensor_copy(u8[:, 2, :], ps_b1[:, 0, :])
                        nc.scalar.activation(
                            u8[:, 3, :], ps_b2[:, 0, :],
                            mybir.ActivationFunctionType.Identity,
                        )
                        u8t[t3] = u8
                        del ent[t3]
                    if 0 <= t4 < NMT:
                        # S4: the out-DMA dispatches an iteration after its
                        # copies completed, so the SP queue never stalls
                        nc.sync.dma_start(
                            out_r[:, :, t4 * MF:(t4 + 1) * MF], u8t.pop(t4),
                        )

    nc.compile()
    return nc


def _get_nc():
    global _CACHED
    if _CACHED is None:
        _CACHED = _build()
    return _CACHED


def kernel(x, proxy, Wq, bq, Wk, bk, Wv, bv, Wo, bo, **run_kwargs):
    nc = _get_nc()

    crow = np.concatenate(
        [np.asarray(bk, np.float32).reshape(1, CK),
         np.asarray(bv, np.float32).reshape(1, CV),
         np.ones((1, 256), np.float32),
         np.asarray(bo, np.float32).reshape(1, C)], axis=1)
    bqbo = np.concatenate(
        [np.asarray(bq, np.float32).reshape(2, P).T,
         np.asarray(bo, np.float32).reshape(4, P).T], axis=1)
    w16 = np.concatenate(
        [np.asarray(Wk).T, np.asarray(Wv).T], axis=1
    ).astype(np.float16)
    shared = {
        "woT": np.ascontiguousarray(Wo.T).astype(np.float16),
        "wq": np.ascontiguousarray(Wq).astype(np.float16),
        "bqs16": np.ascontiguousarray(
            (np.asarray(bq, np.float32) * SCALE).reshape(2, P).T
        ).astype(np.float16),
        "crow": np.ascontiguousarray(crow),
        "bqbo": np.ascontiguousarray(bqbo),
        "ones_col": np.ones((K, 1), np.float32),
    }
    in_maps = []
    for b in range(B):
        m = dict(shared)
        m["x"] = np.ascontiguousarray(x[b]).reshape(C, N).astype(np.float16)
        pf16 = np.asarray(proxy[b, :, :, 0]).astype(np.float16)
        m["pack16"] = np.ascontiguousarray(np.concatenate([pf16, w16], axis=1))
        in_maps.append(m)

    res = bass_utils.run_bass_kernel_spmd(
        nc, in_maps, core_ids=list(range(B)), **run_kwargs
    )
    kernel.last_results = res
    out = np.empty((B, C, N), np.float32)
    for b in range(B):
        u8 = res.results[b]["out"].astype(np.float32)
        sinv = np.asarray(res.results[b]["sinv"], np.float32).reshape(C)
        out[b] = (u8 - 128.0) * sinv[:, None]
    return out.reshape(B, C, H, W)
